# revision 5
# baseline (speedup 1.0000x reference)
"""Trainium2 Bass kernel for sparse (sliding-window, GQA, RoPE) attention.

Sharding: 8-way tensor-parallel over heads. Core c owns q-heads 4c..4c+3 and
kv-head c (wq/wk/wv column-parallel, wo row-parallel); each core produces a
full-shape partial output and the host sums the 8 partials (the all-reduce).

v3 vs v2: the two big projections (QKV and WO) run as fp8-e4m3 DoubleRow
matmuls with a 3-term residual decomposition
    W @ x ~= (W_hi + W_lo) @ x_hi + W_hi @ x_lo
where *_hi = e4m3(t), *_lo = e4m3(t - dequant(t_hi)) share one power-of-two
scale. Each DoubleRow instruction covers two 128-deep k-subtiles at 0.5
cyc/col, so the three terms cost 0.75 cyc/col/k-tile vs 1.0 for bf16 while
keeping ~bf16 precision (residuals are exact-scale floats; PSUM adds slots
1:1). x/w splits are host-prepped; the attention output's hi/lo split runs
on device (DVE mul -> bf16 tmp, Act cast-copy -> hi, Pool subtract -> lo).
Attention itself stays bf16 (softmax amplifies fp8 score noise).
Descale bookkeeping: QK via host-scaled cos/sin tables, V via one scale-AP
activation copy (attn sits at 16x device scale), final 1/(16*Swo) in the
host-side partial sum.
"""
import numpy as np
from contextlib import ExitStack

import ml_dtypes

import concourse.bass as bass
import concourse.bass_isa as bass_isa
from concourse import bacc
import concourse.mybir as mybir
import concourse.tile as tile
from concourse.bass_utils import run_bass_kernel_spmd

BF = mybir.dt.bfloat16
F8 = mybir.dt.float8e4
F32 = mybir.dt.float32
NPBF = ml_dtypes.bfloat16
NPF8 = ml_dtypes.float8_e4m3
DR = mybir.MatmulPerfMode.DoubleRow

NCORE = 8
T = 2048              # total tokens (2 seqs x 1024)
DIM = 4096
SEQ = 1024
NSEQ = 2
HD = 128              # head dim
NH = 4                # q heads per core
NKP = 16              # contraction k-tile PAIRS (32 tiles of 128)
QB = 128              # attention q-block width
SCALE = float(HD) ** -0.5
WCOLS = NH * HD + 2 * HD   # 768 projection output cols per core
ATT_SCALE = 16.0           # device-side scale carried by V/attn

# per-(seq-local qb) score k-tile lists: (seq-local k-tile index, mask id)
# masks: -1 none, 0: causal j>=p, 1: window j<p
QB_TILES = []
for _N in range(8):
    _lo = max(0, _N - 4)
    _tl = []
    for _j in range(_lo, _N + 1):
        _mi = -1
        if _j == _N - 4:
            _mi = 1
        if _j == _N:
            _mi = 0
        _tl.append((_j, _mi))
    QB_TILES.append(_tl)
MAXKT = 5

_NC_CACHE = {}


def _build_nc(reps=1, internal_io=False):
    nc = bacc.Bacc("TRN2", target_bir_lowering=False, debug=False,
                   num_devices=NCORE)
    if internal_io:
        # timing-only variant: big tensors live in device DRAM (no host
        # transfer per run); tiny dummy in/out keep the pjrt contract.
        def dram_in(name, shape, dt):
            return nc.dram_tensor(name, shape, dt).ap()
        dummy_in = nc.declare_dram_parameter("tin", [128, 128], F32, isOutput=False)
        dout = nc.declare_dram_parameter("tout", [128, 128], F32, isOutput=True)
        outp = nc.dram_tensor("outp_i", [T, DIM], BF).ap()
    else:
        def dram_in(name, shape, dt):
            return nc.declare_dram_parameter(name, shape, dt, isOutput=False)
        outp = nc.declare_dram_parameter("outp", [T, DIM], BF, isOutput=True)
    # host pre-arranged layouts (see _host_prep):
    #   xh/xl[ch*128+p, kp*1024 + i*512 + c] = e4m3(Sx * x[ch*512+c,
    #                                               (2kp+i)*128+p]) hi/lo
    #   wh/wl[p, kp*1536 + i*768 + j] = e4m3(Sw * wqkv[j, (2kp+i)*128+p])
    #   woh/wol[p, p_*8192 + e*4096 + o] = e4m3(Swo * wo[o,
    #                                          core*512 + (2p_+e)*128+p])
    xh = dram_in("xh", [4 * 128, NKP * 1024], F8)
    xl = dram_in("xl", [4 * 128, NKP * 1024], F8)
    wh = dram_in("wh", [128, NKP * 2 * WCOLS], F8)
    wl = dram_in("wl", [128, NKP * 2 * WCOLS], F8)
    woh = dram_in("woh", [128, 2 * 2 * DIM], F8)
    wol = dram_in("wol", [128, 2 * 2 * DIM], F8)
    cosT = dram_in("cosT", [128, T], BF)
    sinT = dram_in("sinT", [128, T], BF)
    aux = dram_in("aux", [128, 2 * 2 * QB + 2], BF)
    vsc = dram_in("vsc", [128, 1], F32)

    with tile.TileContext(nc) as tc, ExitStack() as top:
        persist = top.enter_context(tc.tile_pool(name="persist", bufs=1))
        if internal_io:
            # on the Pool SWDGE queue so the SP queue's first transfer is
            # the weights the PE is waiting on
            dtile = persist.tile([128, 32], F32, name="dtile", tag="dtile")
            nc.gpsimd.dma_start(dtile[:], dummy_in[:, 0:32])
            nc.gpsimd.dma_start(dout[:, 0:32], dtile[:])
            nc.gpsimd.dma_start(dout[:, 32:128], dummy_in[:, 32:128])

        aux_sb = persist.tile([128, 2 * 2 * QB + 2], BF, name="aux_sb", tag="aux")
        mask_sb = aux_sb[:, 0:2 * 2 * QB]
        vsc_sb = persist.tile([128, 1], F32, name="vsc_sb", tag="vsc")
        vscale = vsc_sb[:, 0:1]   # ATT_SCALE/(Sx*Sw), fp32 scale AP

        for _rep in range(reps):
         with ExitStack() as rep:
            pq = rep.enter_context(tc.tile_pool(name="pq", bufs=1))
            p1 = rep.enter_context(tc.tile_pool(name="p1", bufs=1))
            p2 = rep.enter_context(tc.tile_pool(name="p2", bufs=1))

            # cross-phase bf16 tiles
            QTps = [[pq.tile([128, 2 * SEQ], BF, name=f"QT{p_}_{s}",
                             tag=f"QT{p_}_{s}") for s in range(2)]
                    for p_ in range(2)]
            KTs = [pq.tile([128, SEQ], BF, name=f"KT{s}", tag=f"KT{s}")
                   for s in range(2)]
            Vc = [pq.tile([128, 512], BF, name=f"Vc{ch}", tag=f"Vc{ch}")
                  for ch in range(4)]
            # attention output, normalized, at ATT_SCALE, e4m3 hi/lo planes
            # laid out [128 feat, tl, head-in-pair, tok] for DoubleRow lhsT
            AH = [[pq.tile([128, 8, 2, 128], F8, name=f"AH{p_}_{s}",
                           tag=f"AH{p_}_{s}") for s in range(2)]
                  for p_ in range(2)]
            AL = [[pq.tile([128, 8, 2, 128], F8, name=f"AL{p_}_{s}",
                           tag=f"AL{p_}_{s}") for s in range(2)]
                  for p_ in range(2)]

            # ---------------- phase 1: projections + rope -----------------
            # A-terms (hi*hi) run first across all groups so the wl/xl
            # streams have a whole A-pass to land; DMA queues are spread:
            # wh+xh(ch>0) on SP, wl+wo on Pool(SWDGE), xh(ch0)+xl+tables
            # on Act.
            wh_sb = p1.tile([128, NKP, 2, WCOLS], F8, name="wh_sb", tag="wh_sb")
            wl_sb = p1.tile([128, NKP, 2, WCOLS], F8, name="wl_sb", tag="wl_sb")
            cos_sb = p1.tile([128, T], BF, name="cos_sb", tag="cos")
            sin_sb = p1.tile([128, T], BF, name="sin_sb", tag="sin")

            def wsl(w_sb, kp, g):
                # stationary [128, 2, 128] for pair kp, output group g
                return w_sb[:, kp, :, g * 128:(g + 1) * 128]

            woh_sb = p2.tile([128, 2, 2, DIM], F8, name="woh_sb", tag="woh_sb")
            wol_sb = p2.tile([128, 2, 2, DIM], F8, name="wol_sb", tag="wol_sb")

            with tc.tile_pool(name="ps1", bufs=1, space="PSUM") as ps1:
                for ch in range(4):
                    s, loc = divmod(ch, 2)
                    csl = slice(ch * 512, (ch + 1) * 512)
                    xhs, xls = [], []
                    r0 = ch * 128
                    for b in range(4):   # blocks of 4 pairs
                        if ch == 0:
                            # wh streams on SP in block-matched pieces; the
                            # very first piece is just pair0/g0 so the PE
                            # starts ~1us earlier
                            if b == 0:
                                nc.sync.dma_start(wh_sb[:, 0:1, :, :],
                                                  wh[:, 0:1536])
                                nc.sync.dma_start(wh_sb[:, 1:4, :, :],
                                                  wh[:, 1536:4 * 1536])
                            else:
                                nc.sync.dma_start(
                                    wh_sb[:, b * 4:(b + 1) * 4, :, :],
                                    wh[:, b * 4 * 1536:(b + 1) * 4 * 1536])
                        th = p1.tile([128, 4, 2, 512], F8, name=f"xh{ch}_{b}",
                                     tag="xhs", bufs=4)
                        xhs.append(th)
                        c0 = b * 4096
                        xq = nc.scalar if ch == 0 else nc.sync
                        if ch == 0 and b == 0:
                            # first pair rides the Pool queue, ahead of wl
                            # (the Act queue starts late behind
                            # LoadActFuncSet; SP must stream wh unimpeded)
                            nc.gpsimd.dma_start(th[:, 0:1, :, :],
                                                xh[r0:r0 + 128, c0:c0 + 1024])
                            xq.dma_start(th[:, 1:4, :, :],
                                         xh[r0:r0 + 128, c0 + 1024:c0 + 4096])
                        else:
                            xq.dma_start(th[:],
                                         xh[r0:r0 + 128, c0:c0 + 4096])
                    for b in range(4):
                        # xl after all of xh: only needed from the C-pass on
                        tl_ = p1.tile([128, 4, 2, 512], F8, name=f"xl{ch}_{b}",
                                      tag="xls", bufs=4)
                        xls.append(tl_)
                        c0 = b * 4096
                        nc.scalar.dma_start(tl_[:],
                                            xl[r0:r0 + 128, c0:c0 + 4096])
                    if ch == 0:
                        # wl on the Pool SWDGE queue, in parallel with wh
                        nc.gpsimd.dma_start(wl_sb[:], wl[:])
                        # deferred small loads: needed only from RoPE time on
                        nc.scalar.dma_start(cos_sb[:], cosT[:])
                        nc.scalar.dma_start(sin_sb[:], sinT[:])
                        if _rep == 0:
                            nc.scalar.dma_start(aux_sb[:], aux[:])
                            nc.scalar.dma_start(vsc_sb[:], vsc[:])
                    if ch == 2:
                        # wo for phase 3, after the Pool queue drains wl
                        nc.gpsimd.dma_start(woh_sb[:], woh[:])
                        nc.gpsimd.dma_start(wol_sb[:], wol[:])
                    accs = [ps1.tile([128, 512], F32, name=f"acc{ch}_{g}",
                                     tag="acc", bufs=8) for g in range(6)]

                    def terms(kp):
                        b, kl = divmod(kp, 4)
                        mh = xhs[b][:, kl, :, :]
                        mlo = xls[b][:, kl, :, :]
                        return ((wh_sb, mh), (wl_sb, mh), (wh_sb, mlo))

                    for ti in (0, 1, 2):
                        for kp in range(NKP if ti == 0 else NKP - 4):
                            w_sb, mv = terms(kp)[ti]
                            for g in range(6):
                                nc.tensor.matmul(
                                    accs[g][:], wsl(w_sb, kp, g), mv,
                                    start=(kp == 0 and ti == 0),
                                    stop=False, perf_mode=DR)
                    # tail: finish each group's last 4 pairs (B/C terms) then
                    # emit its RoPE muls right away, so PSUM accs free
                    # progressively while later groups still accumulate.
                    # sin_sb holds [+sin; -sin] so rotated =
                    # acc*cos + swap_halves(acc*sin_sgn).
                    vtc = p1.tile([128, 512], BF, name="vtc", tag="vtc", bufs=1)
                    for g in range(6):
                        for ti in (1, 2):
                            for kp in range(NKP - 4, NKP):
                                w_sb, mv = terms(kp)[ti]
                                nc.tensor.matmul(
                                    accs[g][:], wsl(w_sb, kp, g), mv,
                                    start=False,
                                    stop=(kp == NKP - 1 and ti == 2),
                                    perf_mode=DR)
                        if g < 4:
                            p_, e = divmod(g, 2)
                            dest = QTps[p_][s][:, loc * 1024 + e:
                                               (loc + 1) * 1024:2]
                        elif g == 4:
                            dest = KTs[s][:, loc * 512:(loc + 1) * 512]
                        else:
                            # V: descale (Sx*Sw) and apply ATT_SCALE in one
                            # per-partition scale-AP copy
                            nc.scalar.activation(
                                vtc[:], accs[5][:],
                                mybir.ActivationFunctionType.Copy,
                                scale=vscale)
                            break
                        # one Act copy PSUM->bf16 releases the acc quickly
                        # (GPSIMD can't read PSUM; DVE PSUM reads are 2x the
                        # cost of bf16 reads), then both rope muls run on
                        # DVE at the cheap 16-bit rate; per-g swap DMA (SP,
                        # clear of the Act queue) then the add on Pool
                        rt = p1.tile([128, 512], BF, name="rt", tag="rt",
                                     bufs=2)
                        bg = p1.tile([128, 512], BF, name="bg", tag="bg",
                                     bufs=2)
                        bs = p1.tile([128, 512], BF, name="bs", tag="bs",
                                     bufs=2)
                        nc.scalar.copy(rt[:], accs[g][:])
                        nc.vector.tensor_mul(bg[:], rt[:], sin_sb[:, csl])
                        nc.vector.tensor_mul(dest, rt[:], cos_sb[:, csl])
                        nc.sync.dma_start(bs[0:64, :], bg[64:128, :])
                        nc.sync.dma_start(bs[64:128, :], bg[0:64, :])
                        # SBUF-only add on the (idle) Pool engine so the DVE
                        # queue is clear for attention's chain at ph1 end
                        nc.gpsimd.tensor_add(dest, dest, bs[:])
                    # V^T -> V via DMA XBAR transpose (no PE / PSUM use)
                    for q4 in range(4):
                        nc.sync.dma_start(Vc[ch][:, q4 * 128:(q4 + 1) * 128],
                                          vtc[:, q4 * 128:(q4 + 1) * 128],
                                          transpose=True)

            # ---------------- phase 2+3: attention + output proj ----------
            with tc.tile_pool(name="psA", bufs=1, space="PSUM") as psA:
                lrows = [[p2.tile([1, 2 * SEQ], BF, name=f"lrow{s}_{p_}",
                                  tag=f"lrow{s}_{p_}", bufs=1)
                          for p_ in range(2)] for s in range(NSEQ)]

                def scores_blk(s, qb):
                    tiles = QB_TILES[qb]
                    n = len(tiles)
                    qsl = slice(2 * qb * QB, 2 * (qb + 1) * QB)
                    pts = []
                    for p_ in range(2):
                        pt = p2.tile([128, MAXKT * 2 * QB], BF, name="pt",
                                     tag="pt", bufs=4)
                        pts.append(pt)
                        for gi in range(0, n, 2):
                            grp = tiles[gi:gi + 2]
                            w_ = len(grp) * 256
                            sc = psA.tile([128, 512], F32, name="sc",
                                          tag="sc", bufs=3)
                            for i, (j, mi) in enumerate(grp):
                                nc.tensor.matmul(
                                    sc[:, i * 256:(i + 1) * 256],
                                    KTs[s][:, j * 128:(j + 1) * 128],
                                    QTps[p_][s][:, qsl],
                                    start=True, stop=True)
                            nc.scalar.activation(
                                pt[:, gi * 256:gi * 256 + w_], sc[:, 0:w_],
                                mybir.ActivationFunctionType.Exp, scale=SCALE)
                        for i, (j, mi) in enumerate(tiles):
                            if mi < 0:
                                continue
                            # SBUF-only, so Pool can own it; keeps DVE clear
                            # for the latency-critical normalize chain
                            nc.gpsimd.tensor_mul(
                                pt[:, i * 256:(i + 1) * 256],
                                pt[:, i * 256:(i + 1) * 256],
                                mask_sb[:, mi * 256:(mi + 1) * 256])
                    return pts

                def pv_blk(s, qb, pts):
                    tiles = QB_TILES[qb]
                    n = len(tiles)
                    ovs = []
                    for p_ in range(2):
                        pt = pts[p_]
                        ov = psA.tile([128, 256], F32, name="ov",
                                      tag="ov", bufs=2)
                        ovs.append(ov)
                        for i, (j, _) in enumerate(tiles):
                            nc.tensor.matmul(
                                ov[:], Vc[s * 2 + j // 4][:, (j % 4) * 128:
                                                          (j % 4 + 1) * 128],
                                pt[:, i * 256:(i + 1) * 256],
                                start=(i == 0), stop=(i == n - 1))
                        # softmax denominator on Pool+DVE instead of PE:
                        # in-place cross-partition sum of the (now dead)
                        # probs, then row-accumulate across k-tiles.
                        nc.gpsimd.partition_all_reduce(
                            pt[:, 0:n * 256], pt[:, 0:n * 256],
                            channels=128, reduce_op=bass_isa.ReduceOp.add)
                        lsl = lrows[s][p_][0:1, qb * 256:(qb + 1) * 256]
                        nc.vector.tensor_copy(lsl, pt[0:1, 0:256])
                        for i in range(1, n):
                            nc.vector.tensor_add(
                                lsl, lsl, pt[0:1, i * 256:(i + 1) * 256])
                    return ovs

                def norm_blk(s, qb, ovs):
                    # denominators are final per block: normalize, then split
                    # into e4m3 hi/lo planes for the DoubleRow projection.
                    # Everything avoids the Act queue: exp latency is the PE
                    # critical path, so Act stays exp-only.
                    for p_ in range(2):
                        lsl = lrows[s][p_][0:1, qb * 256:(qb + 1) * 256]
                        with nc.allow_low_precision(reason="softmax denom scale"):
                            nc.vector.reciprocal(lsl, lsl)
                        lb = p2.tile([128, 256], BF, name="lb", tag="lb",
                                     bufs=2)
                        nc.gpsimd.partition_broadcast(lb[:], lsl)
                        at = p2.tile([128, 256], BF, name="at", tag="at",
                                     bufs=2)
                        nc.vector.tensor_mul(at[:], ovs[p_][:], lb[:])
                        for e in range(2):
                            hi = AH[p_][s][:, qb, e, :]
                            lo = AL[p_][s][:, qb, e, :]
                            nc.scalar.copy(hi, at[:, e::2])
                            nc.gpsimd.tensor_sub(lo, at[:, e::2], hi)

                def proj_tl(s, tl, last_call=False):
                    tb = s * 8 + tl
                    for sh in range(2):
                        stg = p2.tile([128, 2048], BF, name="stg",
                                      tag="stg", bufs=2)
                        last = last_call and sh == 1
                        for cc in range(4):
                            chn = sh * 4 + cc
                            oc = psA.tile([128, 512], F32, name="oc",
                                          tag="oc", bufs=3)
                            ti = 0
                            for p_ in range(2):
                                for stat, mov in ((AH, woh_sb), (AH, wol_sb),
                                                  (AL, woh_sb)):
                                    nc.tensor.matmul(
                                        oc[:],
                                        stat[p_][s][:, tl, :, :],
                                        mov[:, p_, :, chn * 512:
                                            (chn + 1) * 512],
                                        start=(ti == 0), stop=(ti == 5),
                                        perf_mode=DR)
                                    ti += 1
                            dsl = stg[:, cc * 512:(cc + 1) * 512]
                            if cc % 2 == 0:
                                nc.scalar.copy(dsl, oc[:])
                            else:
                                nc.vector.tensor_copy(dsl, oc[:])
                            if last:
                                # drain fast: per-chunk DMAs right after
                                # each copy, alternating queues
                                dq = nc.sync if cc % 2 == 0 else nc.gpsimd
                                dq.dma_start(
                                    outp[tb * 128:(tb + 1) * 128,
                                         chn * 512:(chn + 1) * 512], dsl)
                        if not last:
                            nc.sync.dma_start(
                                outp[tb * 128:(tb + 1) * 128,
                                     sh * 2048:(sh + 1) * 2048], stg[:])

                # per block: scores -> the (qb-2) projection (PE filler
                # while exp/mask run; two blocks of slack keeps the
                # multi-engine normalize/split chain off the PE critical
                # path) -> PV -> normalize+split
                # seq 1 ends with qb0: the final block's norm chain (1
                # k-tile) is the shortest, trimming the drain tail
                order = [(0, qb) for qb in range(SEQ // QB)] \
                    + [(1, qb) for qb in range(1, SEQ // QB)] + [(1, 0)]
                # scores run one block ahead so each block's exp/mask chain
                # has a whole iteration to complete before its PV
                pts_next = scores_blk(*order[0])
                for i, (s, qb) in enumerate(order):
                    pts = pts_next
                    if i + 1 < len(order):
                        pts_next = scores_blk(*order[i + 1])
                    if i >= 2:
                        proj_tl(*order[i - 2])
                    ovs = pv_blk(s, qb, pts)
                    norm_blk(s, qb, ovs)
                proj_tl(*order[-2])
                proj_tl(*order[-1], last_call=True)

    nc.compile()
    return nc


def _get_nc():
    if "nc" not in _NC_CACHE:
        _NC_CACHE["nc"] = _build_nc()
    return _NC_CACHE["nc"]


def _pow2_scale(absmax, target=224.0):
    return 2.0 ** np.floor(np.log2(target / max(absmax, 1e-30)))


def _split8(a, scale):
    """-> (hi, lo) e4m3 planes of a*scale (common power-of-2 scale)."""
    s = (a * scale).astype(np.float32)
    hi = s.astype(NPF8)
    lo = (s - hi.astype(np.float32)).astype(NPF8)
    return hi, lo


def _host_prep(x, cos, sin, wq, wk, wv, wo):
    perm = np.concatenate([np.arange(0, 128, 2), np.arange(1, 128, 2)])
    wq_p = wq.reshape(32, 128, DIM)[:, perm, :].reshape(32 * 128, DIM)
    wk_p = wk.reshape(8, 128, DIM)[:, perm, :].reshape(8 * 128, DIM)
    xT = np.ascontiguousarray(x.T)  # [DIM, T]

    sx = _pow2_scale(np.abs(x).max())
    sw = _pow2_scale(max(np.abs(wq).max(), np.abs(wk).max(), np.abs(wv).max()))
    swo = _pow2_scale(np.abs(wo).max())

    xh_, xl_ = _split8(xT, sx)
    # [DIM, T] -> [ch*128+p, kp*1024 + i*512 + c]
    def xlay(a):
        return np.ascontiguousarray(
            a.reshape(NKP, 2, 128, 4, 512).transpose(3, 2, 0, 1, 4)
            .reshape(4 * 128, NKP * 1024))
    xh_, xl_ = xlay(xh_), xlay(xl_)

    # rope tables absorb 1/(sx*sw)
    dsc = 1.0 / (sx * sw)
    cosT = (np.vstack([cos.T, cos.T]) * dsc).astype(NPBF)
    sinT = (np.vstack([sin.T, -sin.T]) * dsc).astype(NPBF)
    p = np.arange(128)[:, None]
    j = np.arange(QB)[None, :]
    masks = [(j >= p).astype(np.float32), (j < p).astype(np.float32)]
    aux = np.concatenate(
        [np.repeat(m, 2, axis=1) for m in masks]
        + [np.zeros((128, 2), np.float32)], axis=1).astype(NPBF)
    vsc = np.full((128, 1), ATT_SCALE * dsc, np.float32)

    in_maps = []
    for c in range(NCORE):
        wqkv = np.concatenate([
            wq_p[c * 512:(c + 1) * 512],
            wk_p[c * 128:(c + 1) * 128],
            wv[c * 128:(c + 1) * 128]], axis=0)  # [768, DIM]
        whh, wll = _split8(wqkv.T, sw)  # [DIM, 768]

        def wlay(a):
            return np.ascontiguousarray(
                a.reshape(NKP, 2, 128, WCOLS).transpose(2, 0, 1, 3)
                .reshape(128, NKP * 2 * WCOLS))
        wos = wo[:, c * 512:(c + 1) * 512].T  # [512 feat, DIM out]
        woh_, wol_ = _split8(wos, swo)

        def wolay(a):
            return np.ascontiguousarray(
                a.reshape(2, 2, 128, DIM).transpose(2, 0, 1, 3)
                .reshape(128, 2 * 2 * DIM))
        in_maps.append({
            "xh": xh_, "xl": xl_,
            "wh": wlay(whh), "wl": wlay(wll),
            "woh": wolay(woh_), "wol": wolay(wol_),
            "cosT": cosT, "sinT": sinT, "aux": aux, "vsc": vsc,
        })
    return in_maps, 1.0 / (ATT_SCALE * swo)


def kernel(x, cos, sin, wq, wk, wv, wo, n_seqs):
    x = np.asarray(x, dtype=np.float32)
    cos = np.asarray(cos, dtype=np.float32)
    sin = np.asarray(sin, dtype=np.float32)
    wq = np.asarray(wq, dtype=np.float32)
    wk = np.asarray(wk, dtype=np.float32)
    wv = np.asarray(wv, dtype=np.float32)
    wo = np.asarray(wo, dtype=np.float32)
    assert int(n_seqs) == NSEQ and x.shape == (T, DIM)

    nc = _get_nc()
    in_maps, out_dsc = _host_prep(x, cos, sin, wq, wk, wv, wo)
    res = run_bass_kernel_spmd(nc, in_maps, list(range(NCORE))).results
    out = np.zeros((T, DIM), dtype=np.float32)
    for c in range(NCORE):
        out += res[c]["outp"].astype(np.float32)
    return out * out_dsc


# revision 6
# speedup vs baseline: 1.0018x; 1.0018x over previous
"""Trainium2 Bass kernel for sparse (sliding-window, GQA, RoPE) attention.

Sharding: 8-way tensor-parallel over heads. Core c owns q-heads 4c..4c+3 and
kv-head c (wq/wk/wv column-parallel, wo row-parallel); each core produces a
full-shape partial output and the host sums the 8 partials (the all-reduce).

The two big projections (QKV and WO) run as fp8-e4m3 DoubleRow matmuls
with a 3-term residual decomposition
    W @ x ~= (W_hi + W_lo) @ x_hi + W_hi @ x_lo
where *_hi = e4m3(t), *_lo = e4m3(t - dequant(t_hi)) share one power-of-two
scale. Each DoubleRow instruction covers two 128-deep k-subtiles at 0.5
cyc/col, so the three terms cost 0.75 cyc/col/k-tile vs 1.0 for bf16 while
keeping ~bf16 precision (residuals are exact-scale floats; PSUM adds slots
1:1). x/w splits are host-prepped; the attention output's hi/lo split runs
on device (DVE mul -> bf16 tmp, Act cast-copy -> hi, Pool subtract -> lo).
Attention itself stays bf16 (softmax amplifies fp8 score noise; scores
contract over a single 128-wide head_dim so DoubleRow can't pair there).
Descale bookkeeping: QK via host-scaled cos/sin tables, V via one scale-AP
activation copy (attn sits at 16x device scale), final 1/(16*Swo) in the
host-side partial sum.

Schedule: phase 1 streams x hi/lo and runs A-terms (hi*hi) across all six
output groups first so the wl/xl DMA streams have a whole pass to land;
per-group tails emit RoPE (Act PSUM->bf16 copy frees the acc, DVE muls,
SP-queue partition-swap DMA, Pool add) progressively. Phase 2 runs per
128-token block: scores one block AHEAD (exp/mask latency hidden), the
(i-2) block's 48 DoubleRow output-projection matmuls as PE filler, then
PV and the normalize/split chain spread across DVE/Act/Pool. DMA queues:
wh+xh on SP, wl+wo+first-x on Pool SWDGE, xl+tables on Act, output on SP
with the final tile fanned across SP+Pool.
"""
import numpy as np
from contextlib import ExitStack

import ml_dtypes

import concourse.bass as bass
import concourse.bass_isa as bass_isa
from concourse import bacc
import concourse.mybir as mybir
import concourse.tile as tile
from concourse.bass_utils import run_bass_kernel_spmd

BF = mybir.dt.bfloat16
F8 = mybir.dt.float8e4
F32 = mybir.dt.float32
NPBF = ml_dtypes.bfloat16
NPF8 = ml_dtypes.float8_e4m3
DR = mybir.MatmulPerfMode.DoubleRow

NCORE = 8
T = 2048              # total tokens (2 seqs x 1024)
DIM = 4096
SEQ = 1024
NSEQ = 2
HD = 128              # head dim
NH = 4                # q heads per core
NKP = 16              # contraction k-tile PAIRS (32 tiles of 128)
QB = 128              # attention q-block width
SCALE = float(HD) ** -0.5
WCOLS = NH * HD + 2 * HD   # 768 projection output cols per core
ATT_SCALE = 16.0           # device-side scale carried by V/attn

# per-(seq-local qb) score k-tile lists: (seq-local k-tile index, mask id)
# masks: -1 none, 0: causal j>=p, 1: window j<p
QB_TILES = []
for _N in range(8):
    _lo = max(0, _N - 4)
    _tl = []
    for _j in range(_lo, _N + 1):
        _mi = -1
        if _j == _N - 4:
            _mi = 1
        if _j == _N:
            _mi = 0
        _tl.append((_j, _mi))
    QB_TILES.append(_tl)
MAXKT = 5

_NC_CACHE = {}


def _build_nc(reps=1, internal_io=False):
    nc = bacc.Bacc("TRN2", target_bir_lowering=False, debug=False,
                   num_devices=NCORE)
    if internal_io:
        # timing-only variant: big tensors live in device DRAM (no host
        # transfer per run); tiny dummy in/out keep the pjrt contract.
        def dram_in(name, shape, dt):
            return nc.dram_tensor(name, shape, dt).ap()
        dummy_in = nc.declare_dram_parameter("tin", [128, 128], F32, isOutput=False)
        dout = nc.declare_dram_parameter("tout", [128, 128], F32, isOutput=True)
        outp = nc.dram_tensor("outp_i", [T, DIM], BF).ap()
    else:
        def dram_in(name, shape, dt):
            return nc.declare_dram_parameter(name, shape, dt, isOutput=False)
        outp = nc.declare_dram_parameter("outp", [T, DIM], BF, isOutput=True)
    # host pre-arranged layouts (see _host_prep):
    #   xh/xl[ch*128+p, kp*1024 + i*512 + c] = e4m3(Sx * x[ch*512+c,
    #                                               (2kp+i)*128+p]) hi/lo
    #   wh/wl[p, kp*1536 + i*768 + j] = e4m3(Sw * wqkv[j, (2kp+i)*128+p])
    #   woh/wol[p, p_*8192 + e*4096 + o] = e4m3(Swo * wo[o,
    #                                          core*512 + (2p_+e)*128+p])
    xh = dram_in("xh", [4 * 128, NKP * 1024], F8)
    xl = dram_in("xl", [4 * 128, NKP * 1024], F8)
    wh = dram_in("wh", [128, NKP * 2 * WCOLS], F8)
    wl = dram_in("wl", [128, NKP * 2 * WCOLS], F8)
    woh = dram_in("woh", [128, 2 * 2 * DIM], F8)
    wol = dram_in("wol", [128, 2 * 2 * DIM], F8)
    cosT = dram_in("cosT", [128, T], BF)
    sinT = dram_in("sinT", [128, T], BF)
    aux = dram_in("aux", [128, 2 * 2 * QB + 2], BF)
    vsc = dram_in("vsc", [128, 1], F32)

    with tile.TileContext(nc) as tc, ExitStack() as top:
        persist = top.enter_context(tc.tile_pool(name="persist", bufs=1))
        if internal_io:
            # on the Pool SWDGE queue so the SP queue's first transfer is
            # the weights the PE is waiting on
            dtile = persist.tile([128, 32], F32, name="dtile", tag="dtile")
            nc.gpsimd.dma_start(dtile[:], dummy_in[:, 0:32])
            nc.gpsimd.dma_start(dout[:, 0:32], dtile[:])
            nc.gpsimd.dma_start(dout[:, 32:128], dummy_in[:, 32:128])

        aux_sb = persist.tile([128, 2 * 2 * QB + 2], BF, name="aux_sb", tag="aux")
        mask_sb = aux_sb[:, 0:2 * 2 * QB]
        vsc_sb = persist.tile([128, 1], F32, name="vsc_sb", tag="vsc")
        vscale = vsc_sb[:, 0:1]   # ATT_SCALE/(Sx*Sw), fp32 scale AP

        for _rep in range(reps):
         with ExitStack() as rep:
            pq = rep.enter_context(tc.tile_pool(name="pq", bufs=1))
            p1 = rep.enter_context(tc.tile_pool(name="p1", bufs=1))
            p2 = rep.enter_context(tc.tile_pool(name="p2", bufs=1))

            # cross-phase bf16 tiles
            QTps = [[pq.tile([128, 2 * SEQ], BF, name=f"QT{p_}_{s}",
                             tag=f"QT{p_}_{s}") for s in range(2)]
                    for p_ in range(2)]
            KTs = [pq.tile([128, SEQ], BF, name=f"KT{s}", tag=f"KT{s}")
                   for s in range(2)]
            Vc = [pq.tile([128, 512], BF, name=f"Vc{ch}", tag=f"Vc{ch}")
                  for ch in range(4)]
            # attention output, normalized, at ATT_SCALE, e4m3 hi/lo planes
            # laid out [128 feat, tl, head-in-pair, tok] for DoubleRow lhsT
            AH = [[pq.tile([128, 8, 2, 128], F8, name=f"AH{p_}_{s}",
                           tag=f"AH{p_}_{s}") for s in range(2)]
                  for p_ in range(2)]
            AL = [[pq.tile([128, 8, 2, 128], F8, name=f"AL{p_}_{s}",
                           tag=f"AL{p_}_{s}") for s in range(2)]
                  for p_ in range(2)]

            # ---------------- phase 1: projections + rope -----------------
            # A-terms (hi*hi) run first across all groups so the wl/xl
            # streams have a whole A-pass to land; DMA queues are spread:
            # wh+xh(ch>0) on SP, wl+wo on Pool(SWDGE), xh(ch0)+xl+tables
            # on Act.
            wh_sb = p1.tile([128, NKP, 2, WCOLS], F8, name="wh_sb", tag="wh_sb")
            wl_sb = p1.tile([128, NKP, 2, WCOLS], F8, name="wl_sb", tag="wl_sb")
            cos_sb = p1.tile([128, T], BF, name="cos_sb", tag="cos")
            sin_sb = p1.tile([128, T], BF, name="sin_sb", tag="sin")

            def wsl(w_sb, kp, g):
                # stationary [128, 2, 128] for pair kp, output group g
                return w_sb[:, kp, :, g * 128:(g + 1) * 128]

            woh_sb = p2.tile([128, 2, 2, DIM], F8, name="woh_sb", tag="woh_sb")
            wol_sb = p2.tile([128, 2, 2, DIM], F8, name="wol_sb", tag="wol_sb")

            with tc.tile_pool(name="ps1", bufs=1, space="PSUM") as ps1:
                for ch in range(4):
                    s, loc = divmod(ch, 2)
                    csl = slice(ch * 512, (ch + 1) * 512)
                    xhs, xls = [], []
                    r0 = ch * 128
                    for b in range(4):   # blocks of 4 pairs
                        if ch == 0:
                            # wh streams on SP in block-matched pieces; the
                            # very first piece is just pair0/g0 so the PE
                            # starts ~1us earlier
                            if b == 0:
                                nc.sync.dma_start(wh_sb[:, 0:1, :, :],
                                                  wh[:, 0:1536])
                                nc.sync.dma_start(wh_sb[:, 1:4, :, :],
                                                  wh[:, 1536:4 * 1536])
                            else:
                                nc.sync.dma_start(
                                    wh_sb[:, b * 4:(b + 1) * 4, :, :],
                                    wh[:, b * 4 * 1536:(b + 1) * 4 * 1536])
                        th = p1.tile([128, 4, 2, 512], F8, name=f"xh{ch}_{b}",
                                     tag="xhs", bufs=4)
                        xhs.append(th)
                        c0 = b * 4096
                        xq = nc.scalar if ch == 0 else nc.sync
                        if ch == 0 and b == 0:
                            # first pair rides the Pool queue, ahead of wl
                            # (the Act queue starts late behind
                            # LoadActFuncSet; SP must stream wh unimpeded)
                            nc.gpsimd.dma_start(th[:, 0:1, :, :],
                                                xh[r0:r0 + 128, c0:c0 + 1024])
                            xq.dma_start(th[:, 1:4, :, :],
                                         xh[r0:r0 + 128, c0 + 1024:c0 + 4096])
                        else:
                            xq.dma_start(th[:],
                                         xh[r0:r0 + 128, c0:c0 + 4096])
                    for b in range(4):
                        # xl after all of xh: only needed from the C-pass on
                        tl_ = p1.tile([128, 4, 2, 512], F8, name=f"xl{ch}_{b}",
                                      tag="xls", bufs=4)
                        xls.append(tl_)
                        c0 = b * 4096
                        nc.scalar.dma_start(tl_[:],
                                            xl[r0:r0 + 128, c0:c0 + 4096])
                    if ch == 0:
                        # wl on the Pool SWDGE queue, in parallel with wh
                        nc.gpsimd.dma_start(wl_sb[:], wl[:])
                        # deferred small loads: needed only from RoPE time on
                        nc.scalar.dma_start(cos_sb[:], cosT[:])
                        nc.scalar.dma_start(sin_sb[:], sinT[:])
                        if _rep == 0:
                            nc.scalar.dma_start(aux_sb[:], aux[:])
                            nc.scalar.dma_start(vsc_sb[:], vsc[:])
                    if ch == 2:
                        # wo for phase 3, after the Pool queue drains wl
                        nc.gpsimd.dma_start(woh_sb[:], woh[:])
                        nc.gpsimd.dma_start(wol_sb[:], wol[:])
                    accs = [ps1.tile([128, 512], F32, name=f"acc{ch}_{g}",
                                     tag="acc", bufs=8) for g in range(6)]

                    def terms(kp):
                        b, kl = divmod(kp, 4)
                        mh = xhs[b][:, kl, :, :]
                        mlo = xls[b][:, kl, :, :]
                        return ((wh_sb, mh), (wl_sb, mh), (wh_sb, mlo))

                    for ti in (0, 1, 2):
                        for kp in range(NKP if ti == 0 else NKP - 4):
                            w_sb, mv = terms(kp)[ti]
                            for g in range(6):
                                nc.tensor.matmul(
                                    accs[g][:], wsl(w_sb, kp, g), mv,
                                    start=(kp == 0 and ti == 0),
                                    stop=False, perf_mode=DR)
                    # tail: finish each group's last 4 pairs (B/C terms) then
                    # emit its RoPE muls right away, so PSUM accs free
                    # progressively while later groups still accumulate.
                    # sin_sb holds [+sin; -sin] so rotated =
                    # acc*cos + swap_halves(acc*sin_sgn).
                    vtc = p1.tile([128, 512], BF, name="vtc", tag="vtc", bufs=1)
                    for g in range(6):
                        for ti in (1, 2):
                            for kp in range(NKP - 4, NKP):
                                w_sb, mv = terms(kp)[ti]
                                nc.tensor.matmul(
                                    accs[g][:], wsl(w_sb, kp, g), mv,
                                    start=False,
                                    stop=(kp == NKP - 1 and ti == 2),
                                    perf_mode=DR)
                        if g < 4:
                            p_, e = divmod(g, 2)
                            dest = QTps[p_][s][:, loc * 1024 + e:
                                               (loc + 1) * 1024:2]
                        elif g == 4:
                            dest = KTs[s][:, loc * 512:(loc + 1) * 512]
                        else:
                            # V: descale (Sx*Sw) and apply ATT_SCALE in one
                            # per-partition scale-AP copy
                            nc.scalar.activation(
                                vtc[:], accs[5][:],
                                mybir.ActivationFunctionType.Copy,
                                scale=vscale)
                            break
                        # one Act copy PSUM->bf16 releases the acc quickly
                        # (GPSIMD can't read PSUM; DVE PSUM reads are 2x the
                        # cost of bf16 reads), then both rope muls run on
                        # DVE at the cheap 16-bit rate; per-g swap DMA (SP,
                        # clear of the Act queue) then the add on Pool
                        rt = p1.tile([128, 512], BF, name="rt", tag="rt",
                                     bufs=2)
                        bg = p1.tile([128, 512], BF, name="bg", tag="bg",
                                     bufs=2)
                        bs = p1.tile([128, 512], BF, name="bs", tag="bs",
                                     bufs=2)
                        nc.scalar.copy(rt[:], accs[g][:])
                        nc.vector.tensor_mul(bg[:], rt[:], sin_sb[:, csl])
                        nc.vector.tensor_mul(dest, rt[:], cos_sb[:, csl])
                        nc.sync.dma_start(bs[0:64, :], bg[64:128, :])
                        nc.sync.dma_start(bs[64:128, :], bg[0:64, :])
                        # SBUF-only add on the (idle) Pool engine so the DVE
                        # queue is clear for attention's chain at ph1 end
                        nc.gpsimd.tensor_add(dest, dest, bs[:])
                    # V^T -> V via DMA XBAR transpose (no PE / PSUM use)
                    for q4 in range(4):
                        nc.sync.dma_start(Vc[ch][:, q4 * 128:(q4 + 1) * 128],
                                          vtc[:, q4 * 128:(q4 + 1) * 128],
                                          transpose=True)

            # ---------------- phase 2+3: attention + output proj ----------
            with tc.tile_pool(name="psA", bufs=1, space="PSUM") as psA:
                lrows = [[p2.tile([1, 2 * SEQ], BF, name=f"lrow{s}_{p_}",
                                  tag=f"lrow{s}_{p_}", bufs=1)
                          for p_ in range(2)] for s in range(NSEQ)]

                def scores_blk(s, qb):
                    tiles = QB_TILES[qb]
                    n = len(tiles)
                    qsl = slice(2 * qb * QB, 2 * (qb + 1) * QB)
                    pts = []
                    for p_ in range(2):
                        pt = p2.tile([128, MAXKT * 2 * QB], BF, name="pt",
                                     tag="pt", bufs=4)
                        pts.append(pt)
                        for gi in range(0, n, 2):
                            grp = tiles[gi:gi + 2]
                            w_ = len(grp) * 256
                            sc = psA.tile([128, 512], F32, name="sc",
                                          tag="sc", bufs=3)
                            for i, (j, mi) in enumerate(grp):
                                nc.tensor.matmul(
                                    sc[:, i * 256:(i + 1) * 256],
                                    KTs[s][:, j * 128:(j + 1) * 128],
                                    QTps[p_][s][:, qsl],
                                    start=True, stop=True)
                            nc.scalar.activation(
                                pt[:, gi * 256:gi * 256 + w_], sc[:, 0:w_],
                                mybir.ActivationFunctionType.Exp, scale=SCALE)
                        for i, (j, mi) in enumerate(tiles):
                            if mi < 0:
                                continue
                            # SBUF-only, so Pool can own it; keeps DVE clear
                            # for the latency-critical normalize chain
                            nc.gpsimd.tensor_mul(
                                pt[:, i * 256:(i + 1) * 256],
                                pt[:, i * 256:(i + 1) * 256],
                                mask_sb[:, mi * 256:(mi + 1) * 256])
                    return pts

                def pv_blk(s, qb, pts):
                    tiles = QB_TILES[qb]
                    n = len(tiles)
                    ovs = []
                    for p_ in range(2):
                        pt = pts[p_]
                        ov = psA.tile([128, 256], F32, name="ov",
                                      tag="ov", bufs=2)
                        ovs.append(ov)
                        for i, (j, _) in enumerate(tiles):
                            nc.tensor.matmul(
                                ov[:], Vc[s * 2 + j // 4][:, (j % 4) * 128:
                                                          (j % 4 + 1) * 128],
                                pt[:, i * 256:(i + 1) * 256],
                                start=(i == 0), stop=(i == n - 1))
                        # softmax denominator on Pool+DVE instead of PE:
                        # in-place cross-partition sum of the (now dead)
                        # probs, then row-accumulate across k-tiles.
                        nc.gpsimd.partition_all_reduce(
                            pt[:, 0:n * 256], pt[:, 0:n * 256],
                            channels=128, reduce_op=bass_isa.ReduceOp.add)
                        lsl = lrows[s][p_][0:1, qb * 256:(qb + 1) * 256]
                        nc.vector.tensor_copy(lsl, pt[0:1, 0:256])
                        for i in range(1, n):
                            nc.vector.tensor_add(
                                lsl, lsl, pt[0:1, i * 256:(i + 1) * 256])
                    return ovs

                def norm_blk(s, qb, ovs):
                    # denominators are final per block: normalize, then split
                    # into e4m3 hi/lo planes for the DoubleRow projection.
                    # Everything avoids the Act queue: exp latency is the PE
                    # critical path, so Act stays exp-only.
                    for p_ in range(2):
                        lsl = lrows[s][p_][0:1, qb * 256:(qb + 1) * 256]
                        with nc.allow_low_precision(reason="softmax denom scale"):
                            nc.vector.reciprocal(lsl, lsl)
                        lb = p2.tile([128, 256], BF, name="lb", tag="lb",
                                     bufs=2)
                        nc.gpsimd.partition_broadcast(lb[:], lsl)
                        at = p2.tile([128, 256], BF, name="at", tag="at",
                                     bufs=2)
                        nc.vector.tensor_mul(at[:], ovs[p_][:], lb[:])
                        for e in range(2):
                            hi = AH[p_][s][:, qb, e, :]
                            lo = AL[p_][s][:, qb, e, :]
                            nc.scalar.copy(hi, at[:, e::2])
                            nc.gpsimd.tensor_sub(lo, at[:, e::2], hi)

                def proj_tl(s, tl, last_call=False):
                    tb = s * 8 + tl
                    for sh in range(2):
                        stg = p2.tile([128, 2048], BF, name="stg",
                                      tag="stg", bufs=2)
                        last = last_call and sh == 1
                        for cc in range(4):
                            chn = sh * 4 + cc
                            oc = psA.tile([128, 512], F32, name="oc",
                                          tag="oc", bufs=3)
                            ti = 0
                            for p_ in range(2):
                                for stat, mov in ((AH, woh_sb), (AH, wol_sb),
                                                  (AL, woh_sb)):
                                    nc.tensor.matmul(
                                        oc[:],
                                        stat[p_][s][:, tl, :, :],
                                        mov[:, p_, :, chn * 512:
                                            (chn + 1) * 512],
                                        start=(ti == 0), stop=(ti == 5),
                                        perf_mode=DR)
                                    ti += 1
                            dsl = stg[:, cc * 512:(cc + 1) * 512]
                            if cc % 2 == 0:
                                nc.scalar.copy(dsl, oc[:])
                            else:
                                nc.vector.tensor_copy(dsl, oc[:])
                            if last:
                                # drain fast: per-chunk DMAs right after
                                # each copy, alternating queues
                                dq = nc.sync if cc % 2 == 0 else nc.gpsimd
                                dq.dma_start(
                                    outp[tb * 128:(tb + 1) * 128,
                                         chn * 512:(chn + 1) * 512], dsl)
                        if not last:
                            nc.sync.dma_start(
                                outp[tb * 128:(tb + 1) * 128,
                                     sh * 2048:(sh + 1) * 2048], stg[:])

                # per block: scores -> the (qb-2) projection (PE filler
                # while exp/mask run; two blocks of slack keeps the
                # multi-engine normalize/split chain off the PE critical
                # path) -> PV -> normalize+split
                # seq 1 ends with qb0: the final block's norm chain (1
                # k-tile) is the shortest, trimming the drain tail
                order = [(0, qb) for qb in range(SEQ // QB)] \
                    + [(1, qb) for qb in range(1, SEQ // QB)] + [(1, 0)]
                # scores run one block ahead so each block's exp/mask chain
                # has a whole iteration to complete before its PV
                pts_next = scores_blk(*order[0])
                for i, (s, qb) in enumerate(order):
                    pts = pts_next
                    if i + 1 < len(order):
                        pts_next = scores_blk(*order[i + 1])
                    if i >= 2:
                        proj_tl(*order[i - 2])
                    ovs = pv_blk(s, qb, pts)
                    norm_blk(s, qb, ovs)
                proj_tl(*order[-2])
                proj_tl(*order[-1], last_call=True)

    nc.compile()
    return nc


def _get_nc():
    if "nc" not in _NC_CACHE:
        _NC_CACHE["nc"] = _build_nc()
    return _NC_CACHE["nc"]


def _pow2_scale(absmax, target=224.0):
    return 2.0 ** np.floor(np.log2(target / max(absmax, 1e-30)))


def _split8(a, scale):
    """-> (hi, lo) e4m3 planes of a*scale (common power-of-2 scale)."""
    s = (a * scale).astype(np.float32)
    hi = s.astype(NPF8)
    lo = (s - hi.astype(np.float32)).astype(NPF8)
    return hi, lo


def _host_prep(x, cos, sin, wq, wk, wv, wo):
    perm = np.concatenate([np.arange(0, 128, 2), np.arange(1, 128, 2)])
    wq_p = wq.reshape(32, 128, DIM)[:, perm, :].reshape(32 * 128, DIM)
    wk_p = wk.reshape(8, 128, DIM)[:, perm, :].reshape(8 * 128, DIM)
    xT = np.ascontiguousarray(x.T)  # [DIM, T]

    sx = _pow2_scale(np.abs(x).max())
    sw = _pow2_scale(max(np.abs(wq).max(), np.abs(wk).max(), np.abs(wv).max()))
    swo = _pow2_scale(np.abs(wo).max())

    xh_, xl_ = _split8(xT, sx)
    # [DIM, T] -> [ch*128+p, kp*1024 + i*512 + c]
    def xlay(a):
        return np.ascontiguousarray(
            a.reshape(NKP, 2, 128, 4, 512).transpose(3, 2, 0, 1, 4)
            .reshape(4 * 128, NKP * 1024))
    xh_, xl_ = xlay(xh_), xlay(xl_)

    # rope tables absorb 1/(sx*sw)
    dsc = 1.0 / (sx * sw)
    cosT = (np.vstack([cos.T, cos.T]) * dsc).astype(NPBF)
    sinT = (np.vstack([sin.T, -sin.T]) * dsc).astype(NPBF)
    p = np.arange(128)[:, None]
    j = np.arange(QB)[None, :]
    masks = [(j >= p).astype(np.float32), (j < p).astype(np.float32)]
    aux = np.concatenate(
        [np.repeat(m, 2, axis=1) for m in masks]
        + [np.zeros((128, 2), np.float32)], axis=1).astype(NPBF)
    vsc = np.full((128, 1), ATT_SCALE * dsc, np.float32)

    in_maps = []
    for c in range(NCORE):
        wqkv = np.concatenate([
            wq_p[c * 512:(c + 1) * 512],
            wk_p[c * 128:(c + 1) * 128],
            wv[c * 128:(c + 1) * 128]], axis=0)  # [768, DIM]
        whh, wll = _split8(wqkv.T, sw)  # [DIM, 768]

        def wlay(a):
            return np.ascontiguousarray(
                a.reshape(NKP, 2, 128, WCOLS).transpose(2, 0, 1, 3)
                .reshape(128, NKP * 2 * WCOLS))
        wos = wo[:, c * 512:(c + 1) * 512].T  # [512 feat, DIM out]
        woh_, wol_ = _split8(wos, swo)

        def wolay(a):
            return np.ascontiguousarray(
                a.reshape(2, 2, 128, DIM).transpose(2, 0, 1, 3)
                .reshape(128, 2 * 2 * DIM))
        in_maps.append({
            "xh": xh_, "xl": xl_,
            "wh": wlay(whh), "wl": wlay(wll),
            "woh": wolay(woh_), "wol": wolay(wol_),
            "cosT": cosT, "sinT": sinT, "aux": aux, "vsc": vsc,
        })
    return in_maps, 1.0 / (ATT_SCALE * swo)


def kernel(x, cos, sin, wq, wk, wv, wo, n_seqs):
    x = np.asarray(x, dtype=np.float32)
    cos = np.asarray(cos, dtype=np.float32)
    sin = np.asarray(sin, dtype=np.float32)
    wq = np.asarray(wq, dtype=np.float32)
    wk = np.asarray(wk, dtype=np.float32)
    wv = np.asarray(wv, dtype=np.float32)
    wo = np.asarray(wo, dtype=np.float32)
    assert int(n_seqs) == NSEQ and x.shape == (T, DIM)

    nc = _get_nc()
    in_maps, out_dsc = _host_prep(x, cos, sin, wq, wk, wv, wo)
    res = run_bass_kernel_spmd(nc, in_maps, list(range(NCORE))).results
    out = np.zeros((T, DIM), dtype=np.float32)
    for c in range(NCORE):
        out += res[c]["outp"].astype(np.float32)
    return out * out_dsc


# revision 7
# speedup vs baseline: 1.0063x; 1.0045x over previous
"""Trainium2 Bass kernel for sparse (sliding-window, GQA, RoPE) attention.

Sharding: 8-way tensor-parallel over heads. Core c owns q-heads 4c..4c+3 and
kv-head c (wq/wk/wv column-parallel, wo row-parallel); each core produces a
full-shape partial output and the host sums the 8 partials (the all-reduce).

The two big projections (QKV and WO) run as fp8-e4m3 DoubleRow matmuls
with a 3-term residual decomposition
    W @ x ~= (W_hi + W_lo) @ x_hi + W_hi @ x_lo
where *_hi = e4m3(t), *_lo = e4m3(t - dequant(t_hi)) share one power-of-two
scale. Each DoubleRow instruction covers two 128-deep k-subtiles at 0.5
cyc/col, so the three terms cost 0.75 cyc/col/k-tile vs 1.0 for bf16 while
keeping ~bf16 precision (residuals are exact-scale floats; PSUM adds slots
1:1). x/w splits are host-prepped; the attention output's hi/lo split runs
on device (DVE mul -> bf16 tmp, Act cast-copy -> hi, Pool subtract -> lo).
Attention itself stays bf16 (softmax amplifies fp8 score noise; scores
contract over a single 128-wide head_dim so DoubleRow can't pair there).
Descale bookkeeping: QK via host-scaled cos/sin tables, V via one scale-AP
activation copy (attn sits at 16x device scale), final 1/(16*Swo) in the
host-side partial sum.

Schedule: phase 1 streams x hi/lo and runs A-terms (hi*hi) across all six
output groups first so the wl/xl DMA streams have a whole pass to land;
per-group tails emit RoPE (Act PSUM->bf16 copy frees the acc, DVE muls,
SP-queue partition-swap DMA, Pool add) progressively. Phase 2 runs per
128-token block: scores one block AHEAD (exp/mask latency hidden), the
(i-2) block's 48 DoubleRow output-projection matmuls as PE filler, then
PV and the normalize/split chain spread across DVE/Act/Pool. DMA queues:
wh+xh on SP, wl+wo+first-x on Pool SWDGE, xl+tables on Act, output on SP
with the final tile fanned across SP+Pool.
"""
import numpy as np
from contextlib import ExitStack

import ml_dtypes

import concourse.bass as bass
import concourse.bass_isa as bass_isa
from concourse import bacc
import concourse.mybir as mybir
import concourse.tile as tile
from concourse.bass_utils import run_bass_kernel_spmd

BF = mybir.dt.bfloat16
F8 = mybir.dt.float8e4
F32 = mybir.dt.float32
NPBF = ml_dtypes.bfloat16
NPF8 = ml_dtypes.float8_e4m3
DR = mybir.MatmulPerfMode.DoubleRow

NCORE = 8
T = 2048              # total tokens (2 seqs x 1024)
DIM = 4096
SEQ = 1024
NSEQ = 2
HD = 128              # head dim
NH = 4                # q heads per core
NKP = 16              # contraction k-tile PAIRS (32 tiles of 128)
QB = 128              # attention q-block width
SCALE = float(HD) ** -0.5
WCOLS = NH * HD + 2 * HD   # 768 projection output cols per core
ATT_SCALE = 16.0           # device-side scale carried by V/attn

# per-(seq-local qb) score k-tile lists: (seq-local k-tile index, mask id)
# masks: -1 none, 0: causal j>=p, 1: window j<p
QB_TILES = []
for _N in range(8):
    _lo = max(0, _N - 4)
    _tl = []
    for _j in range(_lo, _N + 1):
        _mi = -1
        if _j == _N - 4:
            _mi = 1
        if _j == _N:
            _mi = 0
        _tl.append((_j, _mi))
    QB_TILES.append(_tl)
MAXKT = 5

_NC_CACHE = {}


def _build_nc(reps=1, internal_io=False):
    nc = bacc.Bacc("TRN2", target_bir_lowering=False, debug=False,
                   num_devices=NCORE)
    if internal_io:
        # timing-only variant: big tensors live in device DRAM (no host
        # transfer per run); tiny dummy in/out keep the pjrt contract.
        def dram_in(name, shape, dt):
            return nc.dram_tensor(name, shape, dt).ap()
        dummy_in = nc.declare_dram_parameter("tin", [128, 128], F32, isOutput=False)
        dout = nc.declare_dram_parameter("tout", [128, 128], F32, isOutput=True)
        outp = nc.dram_tensor("outp_i", [T, DIM], BF).ap()
    else:
        def dram_in(name, shape, dt):
            return nc.declare_dram_parameter(name, shape, dt, isOutput=False)
        outp = nc.declare_dram_parameter("outp", [T, DIM], BF, isOutput=True)
    # host pre-arranged layouts (see _host_prep):
    #   xh/xl[ch*128+p, kp*1024 + i*512 + c] = e4m3(Sx * x[ch*512+c,
    #                                               (2kp+i)*128+p]) hi/lo
    #   wh/wl[p, kp*1536 + i*768 + j] = e4m3(Sw * wqkv[j, (2kp+i)*128+p])
    #   woh/wol[p, p_*8192 + e*4096 + o] = e4m3(Swo * wo[o,
    #                                          core*512 + (2p_+e)*128+p])
    xh = dram_in("xh", [4 * 128, NKP * 1024], F8)
    xl = dram_in("xl", [4 * 128, NKP * 1024], F8)
    wh = dram_in("wh", [128, NKP * 2 * WCOLS], F8)
    wl = dram_in("wl", [128, NKP * 2 * WCOLS], F8)
    woh = dram_in("woh", [128, 2 * 2 * DIM], F8)
    wol = dram_in("wol", [128, 2 * 2 * DIM], F8)
    cosT = dram_in("cosT", [128, T], BF)
    sinT = dram_in("sinT", [128, T], BF)
    aux = dram_in("aux", [128, 2 * 2 * QB + 2], BF)
    vsc = dram_in("vsc", [128, 1], F32)

    with tile.TileContext(nc) as tc, ExitStack() as top:
        persist = top.enter_context(tc.tile_pool(name="persist", bufs=1))
        if internal_io:
            # on the Pool SWDGE queue so the SP queue's first transfer is
            # the weights the PE is waiting on
            dtile = persist.tile([128, 32], F32, name="dtile", tag="dtile")
            nc.gpsimd.dma_start(dtile[:], dummy_in[:, 0:32])
            nc.gpsimd.dma_start(dout[:, 0:32], dtile[:])
            nc.gpsimd.dma_start(dout[:, 32:128], dummy_in[:, 32:128])

        aux_sb = persist.tile([128, 2 * 2 * QB + 2], BF, name="aux_sb", tag="aux")
        mask_sb = aux_sb[:, 0:2 * 2 * QB]
        vsc_sb = persist.tile([128, 1], F32, name="vsc_sb", tag="vsc")
        vscale = vsc_sb[:, 0:1]   # ATT_SCALE/(Sx*Sw), fp32 scale AP

        for _rep in range(reps):
         with ExitStack() as rep:
            pq = rep.enter_context(tc.tile_pool(name="pq", bufs=1))
            p1 = rep.enter_context(tc.tile_pool(name="p1", bufs=1))
            p2 = rep.enter_context(tc.tile_pool(name="p2", bufs=1))

            # cross-phase bf16 tiles
            QTps = [[pq.tile([128, 2 * SEQ], BF, name=f"QT{p_}_{s}",
                             tag=f"QT{p_}_{s}") for s in range(2)]
                    for p_ in range(2)]
            KTs = [pq.tile([128, SEQ], BF, name=f"KT{s}", tag=f"KT{s}")
                   for s in range(2)]
            Vc = [pq.tile([128, 512], BF, name=f"Vc{ch}", tag=f"Vc{ch}")
                  for ch in range(4)]
            # attention output, normalized, at ATT_SCALE, e4m3 hi/lo planes
            # laid out [128 feat, tl, head-in-pair, tok] for DoubleRow lhsT
            AH = [[pq.tile([128, 8, 2, 128], F8, name=f"AH{p_}_{s}",
                           tag=f"AH{p_}_{s}") for s in range(2)]
                  for p_ in range(2)]
            AL = [[pq.tile([128, 8, 2, 128], F8, name=f"AL{p_}_{s}",
                           tag=f"AL{p_}_{s}") for s in range(2)]
                  for p_ in range(2)]

            # ---------------- phase 1: projections + rope -----------------
            # A-terms (hi*hi) run first across all groups so the wl/xl
            # streams have a whole A-pass to land; DMA queues are spread:
            # wh+xh(ch>0) on SP, wl+wo on Pool(SWDGE), xh(ch0)+xl+tables
            # on Act.
            # [kp][g][i][128] so the first DMA can be just pair0/g0 (256B)
            wh_sb = p1.tile([128, NKP, 6, 2, 128], F8, name="wh_sb",
                            tag="wh_sb")
            wl_sb = p1.tile([128, NKP, 6, 2, 128], F8, name="wl_sb",
                            tag="wl_sb")
            cos_sb = p1.tile([128, T], BF, name="cos_sb", tag="cos")
            sin_sb = p1.tile([128, T], BF, name="sin_sb", tag="sin")

            def wsl(w_sb, kp, g):
                # stationary [128, 2, 128] for pair kp, output group g
                return w_sb[:, kp, g, :, :]

            woh_sb = p2.tile([128, 2, 2, DIM], F8, name="woh_sb", tag="woh_sb")
            wol_sb = p2.tile([128, 2, 2, DIM], F8, name="wol_sb", tag="wol_sb")

            def scores_blk(s, qb, scpool, sctag, scbufs):
                tiles = QB_TILES[qb]
                n = len(tiles)
                qsl = slice(2 * qb * QB, 2 * (qb + 1) * QB)
                pts = []
                for p_ in range(2):
                    pt = p2.tile([128, MAXKT * 2 * QB], BF, name="pt",
                                 tag="pt", bufs=4)
                    pts.append(pt)
                    for gi in range(0, n, 2):
                        grp = tiles[gi:gi + 2]
                        w_ = len(grp) * 256
                        sc = scpool.tile([128, 512], F32, name="sc",
                                         tag=sctag, bufs=scbufs)
                        for i, (j, mi) in enumerate(grp):
                            nc.tensor.matmul(
                                sc[:, i * 256:(i + 1) * 256],
                                KTs[s][:, j * 128:(j + 1) * 128],
                                QTps[p_][s][:, qsl],
                                start=True, stop=True)
                        nc.scalar.activation(
                            pt[:, gi * 256:gi * 256 + w_], sc[:, 0:w_],
                            mybir.ActivationFunctionType.Exp, scale=SCALE)
                    for i, (j, mi) in enumerate(tiles):
                        if mi < 0:
                            continue
                        # SBUF-only, so Pool can own it; keeps DVE clear
                        # for the latency-critical normalize chain
                        nc.gpsimd.tensor_mul(
                            pt[:, i * 256:(i + 1) * 256],
                            pt[:, i * 256:(i + 1) * 256],
                            mask_sb[:, mi * 256:(mi + 1) * 256])
                return pts

            with tc.tile_pool(name="ps1", bufs=1, space="PSUM") as ps1:
                for ch in range(4):
                    s, loc = divmod(ch, 2)
                    csl = slice(ch * 512, (ch + 1) * 512)
                    xhs, xls = [], []
                    r0 = ch * 128
                    for b in range(4):   # blocks of 4 pairs
                        if ch == 0:
                            # wh streams on SP in block-matched pieces; the
                            # very first piece is just pair0/g0 so the PE
                            # starts ~0.5us earlier
                            if b == 0:
                                nc.sync.dma_start(wh_sb[:, 0:1, :, :, :],
                                                  wh[:, 0:1536])
                                nc.sync.dma_start(wh_sb[:, 1:4, :, :, :],
                                                  wh[:, 1536:4 * 1536])
                            else:
                                nc.sync.dma_start(
                                    wh_sb[:, b * 4:(b + 1) * 4, :, :, :],
                                    wh[:, b * 4 * 1536:(b + 1) * 4 * 1536])
                        th = p1.tile([128, 4, 2, 512], F8, name=f"xh{ch}_{b}",
                                     tag="xhs", bufs=4)
                        xhs.append(th)
                        c0 = b * 4096
                        xq = nc.scalar if ch == 0 else nc.sync
                        if ch == 0 and b == 0:
                            # first pair rides the Pool queue, ahead of wl
                            # (the Act queue starts late behind
                            # LoadActFuncSet; SP must stream wh unimpeded)
                            nc.gpsimd.dma_start(th[:, 0:1, :, :],
                                                xh[r0:r0 + 128, c0:c0 + 1024])
                            xq.dma_start(th[:, 1:4, :, :],
                                         xh[r0:r0 + 128, c0 + 1024:c0 + 4096])
                        else:
                            xq.dma_start(th[:],
                                         xh[r0:r0 + 128, c0:c0 + 4096])
                    for b in range(4):
                        # xl after all of xh: only needed from the C-pass on
                        tl_ = p1.tile([128, 4, 2, 512], F8, name=f"xl{ch}_{b}",
                                      tag="xls", bufs=4)
                        xls.append(tl_)
                        c0 = b * 4096
                        nc.scalar.dma_start(tl_[:],
                                            xl[r0:r0 + 128, c0:c0 + 4096])
                    if ch == 0:
                        # wl on the Pool SWDGE queue, in parallel with wh
                        nc.gpsimd.dma_start(wl_sb[:], wl[:])
                        # deferred small loads: needed only from RoPE time on
                        nc.scalar.dma_start(cos_sb[:], cosT[:])
                        nc.scalar.dma_start(sin_sb[:], sinT[:])
                        if _rep == 0:
                            nc.scalar.dma_start(aux_sb[:], aux[:])
                            nc.scalar.dma_start(vsc_sb[:], vsc[:])
                    if ch == 2:
                        # wo for phase 3, after the Pool queue drains wl
                        nc.gpsimd.dma_start(woh_sb[:], woh[:])
                        nc.gpsimd.dma_start(wol_sb[:], wol[:])
                    accs = [ps1.tile([128, 512], F32, name=f"acc{ch}_{g}",
                                     tag="acc", bufs=8) for g in range(6)]

                    def terms(kp):
                        b, kl = divmod(kp, 4)
                        mh = xhs[b][:, kl, :, :]
                        mlo = xls[b][:, kl, :, :]
                        return ((wh_sb, mh), (wl_sb, mh), (wh_sb, mlo))

                    for ti in (0, 1, 2):
                        for kp in range(NKP if ti == 0 else NKP - 4):
                            w_sb, mv = terms(kp)[ti]
                            for g in range(6):
                                nc.tensor.matmul(
                                    accs[g][:], wsl(w_sb, kp, g), mv,
                                    start=(kp == 0 and ti == 0),
                                    stop=False, perf_mode=DR)
                    # tail: finish each group's last 4 pairs (B/C terms) then
                    # emit its RoPE muls right away, so PSUM accs free
                    # progressively while later groups still accumulate.
                    # sin_sb holds [+sin; -sin] so rotated =
                    # acc*cos + swap_halves(acc*sin_sgn).
                    vtc = p1.tile([128, 512], BF, name="vtc", tag="vtc", bufs=1)
                    if ch == 3:
                        # first two blocks' scores live INSIDE the ps1 pool
                        # (acc-tag rotation slots), emitted before the tail:
                        # their exps head the Act queue, their matmuls keep
                        # the PE rolling across the ps1->psA close barrier
                        pre = [scores_blk(0, 0, ps1, "acc", 8),
                               scores_blk(0, 1, ps1, "acc", 8)]
                    for g in range(6):
                        for ti in (1, 2):
                            for kp in range(NKP - 4, NKP):
                                w_sb, mv = terms(kp)[ti]
                                nc.tensor.matmul(
                                    accs[g][:], wsl(w_sb, kp, g), mv,
                                    start=False,
                                    stop=(kp == NKP - 1 and ti == 2),
                                    perf_mode=DR)
                        if g < 4:
                            p_, e = divmod(g, 2)
                            dest = QTps[p_][s][:, loc * 1024 + e:
                                               (loc + 1) * 1024:2]
                        elif g == 4:
                            dest = KTs[s][:, loc * 512:(loc + 1) * 512]
                        else:
                            # V: descale (Sx*Sw) and apply ATT_SCALE in one
                            # per-partition scale-AP copy
                            nc.scalar.activation(
                                vtc[:], accs[5][:],
                                mybir.ActivationFunctionType.Copy,
                                scale=vscale)
                            break
                        # one Act copy PSUM->bf16 releases the acc quickly
                        # (GPSIMD can't read PSUM; DVE PSUM reads are 2x the
                        # cost of bf16 reads), then both rope muls run on
                        # DVE at the cheap 16-bit rate; per-g swap DMA (SP,
                        # clear of the Act queue) then the add on Pool
                        rt = p1.tile([128, 512], BF, name="rt", tag="rt",
                                     bufs=2)
                        bg = p1.tile([128, 512], BF, name="bg", tag="bg",
                                     bufs=2)
                        bs = p1.tile([128, 512], BF, name="bs", tag="bs",
                                     bufs=2)
                        # alternate engines so the serial per-tail chain
                        # (which gates acc release and the ph1->ph2 pool
                        # barrier) is split across Act and DVE
                        if g % 2 == 0:
                            nc.scalar.copy(rt[:], accs[g][:])
                        else:
                            nc.vector.tensor_copy(rt[:], accs[g][:])
                        nc.vector.tensor_mul(bg[:], rt[:], sin_sb[:, csl])
                        nc.vector.tensor_mul(dest, rt[:], cos_sb[:, csl])
                        nc.sync.dma_start(bs[0:64, :], bg[64:128, :])
                        nc.sync.dma_start(bs[64:128, :], bg[0:64, :])
                        # SBUF-only add on the (idle) Pool engine so the DVE
                        # queue is clear for attention's chain at ph1 end
                        nc.gpsimd.tensor_add(dest, dest, bs[:])
                    # V^T -> V via DMA XBAR transpose (no PE / PSUM use)
                    for q4 in range(4):
                        nc.sync.dma_start(Vc[ch][:, q4 * 128:(q4 + 1) * 128],
                                          vtc[:, q4 * 128:(q4 + 1) * 128],
                                          transpose=True)
            # ---------------- phase 2+3: attention + output proj ----------
            with tc.tile_pool(name="psA", bufs=1, space="PSUM") as psA:
                lrows = [[p2.tile([1, 2 * SEQ], BF, name=f"lrow{s}_{p_}",
                                  tag=f"lrow{s}_{p_}", bufs=1)
                          for p_ in range(2)] for s in range(NSEQ)]

                def pv_blk(s, qb, pts):
                    tiles = QB_TILES[qb]
                    n = len(tiles)
                    ovs = []
                    for p_ in range(2):
                        pt = pts[p_]
                        ov = psA.tile([128, 256], F32, name="ov",
                                      tag="ov", bufs=2)
                        ovs.append(ov)
                        for i, (j, _) in enumerate(tiles):
                            nc.tensor.matmul(
                                ov[:], Vc[s * 2 + j // 4][:, (j % 4) * 128:
                                                          (j % 4 + 1) * 128],
                                pt[:, i * 256:(i + 1) * 256],
                                start=(i == 0), stop=(i == n - 1))
                        # softmax denominator on Pool+DVE instead of PE:
                        # in-place cross-partition sum of the (now dead)
                        # probs, then row-accumulate across k-tiles.
                        nc.gpsimd.partition_all_reduce(
                            pt[:, 0:n * 256], pt[:, 0:n * 256],
                            channels=128, reduce_op=bass_isa.ReduceOp.add)
                        lsl = lrows[s][p_][0:1, qb * 256:(qb + 1) * 256]
                        nc.vector.tensor_copy(lsl, pt[0:1, 0:256])
                        for i in range(1, n):
                            nc.vector.tensor_add(
                                lsl, lsl, pt[0:1, i * 256:(i + 1) * 256])
                    return ovs

                def norm_blk(s, qb, ovs):
                    # denominators are final per block: normalize, then split
                    # into e4m3 hi/lo planes for the DoubleRow projection.
                    # Everything avoids the Act queue: exp latency is the PE
                    # critical path, so Act stays exp-only.
                    for p_ in range(2):
                        lsl = lrows[s][p_][0:1, qb * 256:(qb + 1) * 256]
                        with nc.allow_low_precision(reason="softmax denom scale"):
                            nc.vector.reciprocal(lsl, lsl)
                        lb = p2.tile([128, 256], BF, name="lb", tag="lb",
                                     bufs=2)
                        nc.gpsimd.partition_broadcast(lb[:], lsl)
                        at = p2.tile([128, 256], BF, name="at", tag="at",
                                     bufs=2)
                        nc.vector.tensor_mul(at[:], ovs[p_][:], lb[:])
                        for e in range(2):
                            hi = AH[p_][s][:, qb, e, :]
                            lo = AL[p_][s][:, qb, e, :]
                            nc.scalar.copy(hi, at[:, e::2])
                            nc.gpsimd.tensor_sub(lo, at[:, e::2], hi)

                def proj_tl(s, tl, last_call=False):
                    tb = s * 8 + tl
                    for sh in range(2):
                        stg = p2.tile([128, 2048], BF, name="stg",
                                      tag="stg", bufs=2)
                        last = last_call and sh == 1
                        for cc in range(4):
                            chn = sh * 4 + cc
                            oc = psA.tile([128, 512], F32, name="oc",
                                          tag="oc", bufs=3)
                            ti = 0
                            for p_ in range(2):
                                for stat, mov in ((AH, woh_sb), (AH, wol_sb),
                                                  (AL, woh_sb)):
                                    nc.tensor.matmul(
                                        oc[:],
                                        stat[p_][s][:, tl, :, :],
                                        mov[:, p_, :, chn * 512:
                                            (chn + 1) * 512],
                                        start=(ti == 0), stop=(ti == 5),
                                        perf_mode=DR)
                                    ti += 1
                            dsl = stg[:, cc * 512:(cc + 1) * 512]
                            if cc % 2 == 0:
                                nc.scalar.copy(dsl, oc[:])
                            else:
                                nc.vector.tensor_copy(dsl, oc[:])
                            if last:
                                # drain fast: per-chunk DMAs right after
                                # each copy, alternating queues
                                dq = nc.sync if cc % 2 == 0 else nc.gpsimd
                                dq.dma_start(
                                    outp[tb * 128:(tb + 1) * 128,
                                         chn * 512:(chn + 1) * 512], dsl)
                        if not last:
                            nc.sync.dma_start(
                                outp[tb * 128:(tb + 1) * 128,
                                     sh * 2048:(sh + 1) * 2048], stg[:])

                # per block: scores -> the (qb-2) projection (PE filler
                # while exp/mask run; two blocks of slack keeps the
                # multi-engine normalize/split chain off the PE critical
                # path) -> PV -> normalize+split
                # seq 1 ends with qb0: the final block's norm chain (1
                # k-tile) is the shortest, trimming the drain tail
                order = [(0, qb) for qb in range(SEQ // QB)] \
                    + [(1, qb) for qb in range(1, SEQ // QB)] + [(1, 0)]
                # scores run one block ahead so each block's exp/mask chain
                # has a whole iteration to complete before its PV; blocks 0
                # and 1 were pre-emitted inside the ps1 pool
                pts_next = None
                for i, (s, qb) in enumerate(order):
                    pts = pre[i] if i < 2 else pts_next
                    if 1 <= i < len(order) - 1:
                        pts_next = scores_blk(*order[i + 1], psA, "sc", 3)
                    if i >= 2:
                        proj_tl(*order[i - 2])
                    ovs = pv_blk(s, qb, pts)
                    norm_blk(s, qb, ovs)
                proj_tl(*order[-2])
                proj_tl(*order[-1], last_call=True)

    nc.compile()
    return nc


def _get_nc():
    if "nc" not in _NC_CACHE:
        _NC_CACHE["nc"] = _build_nc()
    return _NC_CACHE["nc"]


def _pow2_scale(absmax, target=224.0):
    return 2.0 ** np.floor(np.log2(target / max(absmax, 1e-30)))


def _split8(a, scale):
    """-> (hi, lo) e4m3 planes of a*scale (common power-of-2 scale)."""
    s = (a * scale).astype(np.float32)
    hi = s.astype(NPF8)
    lo = (s - hi.astype(np.float32)).astype(NPF8)
    return hi, lo


def _host_prep(x, cos, sin, wq, wk, wv, wo):
    perm = np.concatenate([np.arange(0, 128, 2), np.arange(1, 128, 2)])
    wq_p = wq.reshape(32, 128, DIM)[:, perm, :].reshape(32 * 128, DIM)
    wk_p = wk.reshape(8, 128, DIM)[:, perm, :].reshape(8 * 128, DIM)
    xT = np.ascontiguousarray(x.T)  # [DIM, T]

    sx = _pow2_scale(np.abs(x).max())
    sw = _pow2_scale(max(np.abs(wq).max(), np.abs(wk).max(), np.abs(wv).max()))
    swo = _pow2_scale(np.abs(wo).max())

    xh_, xl_ = _split8(xT, sx)
    # [DIM, T] -> [ch*128+p, kp*1024 + i*512 + c]
    def xlay(a):
        return np.ascontiguousarray(
            a.reshape(NKP, 2, 128, 4, 512).transpose(3, 2, 0, 1, 4)
            .reshape(4 * 128, NKP * 1024))
    xh_, xl_ = xlay(xh_), xlay(xl_)

    # rope tables absorb 1/(sx*sw)
    dsc = 1.0 / (sx * sw)
    cosT = (np.vstack([cos.T, cos.T]) * dsc).astype(NPBF)
    sinT = (np.vstack([sin.T, -sin.T]) * dsc).astype(NPBF)
    p = np.arange(128)[:, None]
    j = np.arange(QB)[None, :]
    masks = [(j >= p).astype(np.float32), (j < p).astype(np.float32)]
    aux = np.concatenate(
        [np.repeat(m, 2, axis=1) for m in masks]
        + [np.zeros((128, 2), np.float32)], axis=1).astype(NPBF)
    vsc = np.full((128, 1), ATT_SCALE * dsc, np.float32)

    in_maps = []
    for c in range(NCORE):
        wqkv = np.concatenate([
            wq_p[c * 512:(c + 1) * 512],
            wk_p[c * 128:(c + 1) * 128],
            wv[c * 128:(c + 1) * 128]], axis=0)  # [768, DIM]
        whh, wll = _split8(wqkv.T, sw)  # [DIM, 768]

        def wlay(a):
            # [p, kp, g, i, m]: group-major within a pair so the first DMA
            # can deliver just pair0/g0
            return np.ascontiguousarray(
                a.reshape(NKP, 2, 128, 6, 128).transpose(2, 0, 3, 1, 4)
                .reshape(128, NKP * 2 * WCOLS))
        wos = wo[:, c * 512:(c + 1) * 512].T  # [512 feat, DIM out]
        woh_, wol_ = _split8(wos, swo)

        def wolay(a):
            return np.ascontiguousarray(
                a.reshape(2, 2, 128, DIM).transpose(2, 0, 1, 3)
                .reshape(128, 2 * 2 * DIM))
        in_maps.append({
            "xh": xh_, "xl": xl_,
            "wh": wlay(whh), "wl": wlay(wll),
            "woh": wolay(woh_), "wol": wolay(wol_),
            "cosT": cosT, "sinT": sinT, "aux": aux, "vsc": vsc,
        })
    return in_maps, 1.0 / (ATT_SCALE * swo)


def kernel(x, cos, sin, wq, wk, wv, wo, n_seqs):
    x = np.asarray(x, dtype=np.float32)
    cos = np.asarray(cos, dtype=np.float32)
    sin = np.asarray(sin, dtype=np.float32)
    wq = np.asarray(wq, dtype=np.float32)
    wk = np.asarray(wk, dtype=np.float32)
    wv = np.asarray(wv, dtype=np.float32)
    wo = np.asarray(wo, dtype=np.float32)
    assert int(n_seqs) == NSEQ and x.shape == (T, DIM)

    nc = _get_nc()
    in_maps, out_dsc = _host_prep(x, cos, sin, wq, wk, wv, wo)
    res = run_bass_kernel_spmd(nc, in_maps, list(range(NCORE))).results
    out = np.zeros((T, DIM), dtype=np.float32)
    for c in range(NCORE):
        out += res[c]["outp"].astype(np.float32)
    return out * out_dsc


# revision 8
# speedup vs baseline: 1.0066x; 1.0003x over previous
"""Trainium2 Bass kernel for sparse (sliding-window, GQA, RoPE) attention.

Sharding: 8-way tensor-parallel over heads. Core c owns q-heads 4c..4c+3 and
kv-head c (wq/wk/wv column-parallel, wo row-parallel); each core produces a
full-shape partial output and the host sums the 8 partials (the all-reduce).

The two big projections (QKV and WO) run as fp8-e4m3 DoubleRow matmuls
with a 3-term residual decomposition
    W @ x ~= (W_hi + W_lo) @ x_hi + W_hi @ x_lo
where *_hi = e4m3(t), *_lo = e4m3(t - dequant(t_hi)) share one power-of-two
scale. Each DoubleRow instruction covers two 128-deep k-subtiles at 0.5
cyc/col, so the three terms cost 0.75 cyc/col/k-tile vs 1.0 for bf16 while
keeping ~bf16 precision (residuals are exact-scale floats; PSUM adds slots
1:1). x/w splits are host-prepped; the attention output's hi/lo split runs
on device (DVE mul -> bf16 tmp, Act cast-copy -> hi, Pool subtract -> lo).
Attention itself stays bf16 (softmax amplifies fp8 score noise; scores
contract over a single 128-wide head_dim so DoubleRow can't pair there).
Descale bookkeeping: QK via host-scaled cos/sin tables, V via one scale-AP
activation copy (attn sits at 16x device scale), final 1/(16*Swo) in the
host-side partial sum.

Schedule: phase 1 streams x hi/lo and runs A-terms (hi*hi) across all six
output groups first so the wl/xl DMA streams have a whole pass to land;
per-group tails emit RoPE (Act PSUM->bf16 copy frees the acc, DVE muls,
SP-queue partition-swap DMA, Pool add) progressively. Phase 2 runs per
128-token block: scores one block AHEAD (exp/mask latency hidden), the
(i-2) block's 48 DoubleRow output-projection matmuls as PE filler, then
PV and the normalize/split chain spread across DVE/Act/Pool. DMA queues:
wh+xh on SP, wl+wo+first-x on Pool SWDGE, xl+tables on Act, output on SP
with the final tile fanned across SP+Pool.
"""
import numpy as np
from contextlib import ExitStack

import ml_dtypes

import concourse.bass as bass
import concourse.bass_isa as bass_isa
from concourse import bacc
import concourse.mybir as mybir
import concourse.tile as tile
from concourse.bass_utils import run_bass_kernel_spmd

BF = mybir.dt.bfloat16
F8 = mybir.dt.float8e4
F32 = mybir.dt.float32
NPBF = ml_dtypes.bfloat16
NPF8 = ml_dtypes.float8_e4m3
DR = mybir.MatmulPerfMode.DoubleRow

NCORE = 8
T = 2048              # total tokens (2 seqs x 1024)
DIM = 4096
SEQ = 1024
NSEQ = 2
HD = 128              # head dim
NH = 4                # q heads per core
NKP = 16              # contraction k-tile PAIRS (32 tiles of 128)
QB = 128              # attention q-block width
SCALE = float(HD) ** -0.5
WCOLS = NH * HD + 2 * HD   # 768 projection output cols per core
ATT_SCALE = 16.0           # device-side scale carried by V/attn

# per-(seq-local qb) score k-tile lists: (seq-local k-tile index, mask id)
# masks: -1 none, 0: causal j>=p, 1: window j<p
QB_TILES = []
for _N in range(8):
    _lo = max(0, _N - 4)
    _tl = []
    for _j in range(_lo, _N + 1):
        _mi = -1
        if _j == _N - 4:
            _mi = 1
        if _j == _N:
            _mi = 0
        _tl.append((_j, _mi))
    QB_TILES.append(_tl)
MAXKT = 5

_NC_CACHE = {}


def _build_nc(reps=1, internal_io=False):
    nc = bacc.Bacc("TRN2", target_bir_lowering=False, debug=False,
                   num_devices=NCORE)
    if internal_io:
        # timing-only variant: big tensors live in device DRAM (no host
        # transfer per run); tiny dummy in/out keep the pjrt contract.
        def dram_in(name, shape, dt):
            return nc.dram_tensor(name, shape, dt).ap()
        dummy_in = nc.declare_dram_parameter("tin", [128, 128], F32, isOutput=False)
        dout = nc.declare_dram_parameter("tout", [128, 128], F32, isOutput=True)
        outp = nc.dram_tensor("outp_i", [T, DIM], BF).ap()
    else:
        def dram_in(name, shape, dt):
            return nc.declare_dram_parameter(name, shape, dt, isOutput=False)
        outp = nc.declare_dram_parameter("outp", [T, DIM], BF, isOutput=True)
    # host pre-arranged layouts (see _host_prep):
    #   xh/xl[ch*128+p, kp*1024 + i*512 + c] = e4m3(Sx * x[ch*512+c,
    #                                               (2kp+i)*128+p]) hi/lo
    #   wh/wl[p, kp*1536 + i*768 + j] = e4m3(Sw * wqkv[j, (2kp+i)*128+p])
    #   woh/wol[p, p_*8192 + e*4096 + o] = e4m3(Swo * wo[o,
    #                                          core*512 + (2p_+e)*128+p])
    xh = dram_in("xh", [4 * 128, NKP * 1024], F8)
    xl = dram_in("xl", [4 * 128, NKP * 1024], F8)
    wh = dram_in("wh", [128, NKP * 2 * WCOLS], F8)
    wl = dram_in("wl", [128, NKP * 2 * WCOLS], F8)
    woh = dram_in("woh", [128, 2 * 2 * DIM], F8)
    wol = dram_in("wol", [128, 2 * 2 * DIM], F8)
    cosT = dram_in("cosT", [128, T], BF)
    sinT = dram_in("sinT", [128, T], BF)
    aux = dram_in("aux", [128, 2 * 2 * QB + 2], BF)
    vsc = dram_in("vsc", [128, 1], F32)

    with tile.TileContext(nc) as tc, ExitStack() as top:
        persist = top.enter_context(tc.tile_pool(name="persist", bufs=1))
        if internal_io:
            # on the Pool SWDGE queue so the SP queue's first transfer is
            # the weights the PE is waiting on
            dtile = persist.tile([128, 32], F32, name="dtile", tag="dtile")
            nc.gpsimd.dma_start(dtile[:], dummy_in[:, 0:32])
            nc.gpsimd.dma_start(dout[:, 0:32], dtile[:])
            nc.gpsimd.dma_start(dout[:, 32:128], dummy_in[:, 32:128])

        aux_sb = persist.tile([128, 2 * 2 * QB + 2], BF, name="aux_sb", tag="aux")
        mask_sb = aux_sb[:, 0:2 * 2 * QB]
        vsc_sb = persist.tile([128, 1], F32, name="vsc_sb", tag="vsc")
        vscale = vsc_sb[:, 0:1]   # ATT_SCALE/(Sx*Sw), fp32 scale AP

        for _rep in range(reps):
         with ExitStack() as rep:
            pq = rep.enter_context(tc.tile_pool(name="pq", bufs=1))
            p1 = rep.enter_context(tc.tile_pool(name="p1", bufs=1))
            p2 = rep.enter_context(tc.tile_pool(name="p2", bufs=1))

            # cross-phase bf16 tiles
            QTps = [[pq.tile([128, 2 * SEQ], BF, name=f"QT{p_}_{s}",
                             tag=f"QT{p_}_{s}") for s in range(2)]
                    for p_ in range(2)]
            KTs = [pq.tile([128, SEQ], BF, name=f"KT{s}", tag=f"KT{s}")
                   for s in range(2)]
            Vc = [pq.tile([128, 512], BF, name=f"Vc{ch}", tag=f"Vc{ch}")
                  for ch in range(4)]
            # attention output, normalized, at ATT_SCALE, e4m3 hi/lo planes
            # laid out [128 feat, tl, head-in-pair, tok] for DoubleRow lhsT
            AH = [[pq.tile([128, 8, 2, 128], F8, name=f"AH{p_}_{s}",
                           tag=f"AH{p_}_{s}") for s in range(2)]
                  for p_ in range(2)]
            AL = [[pq.tile([128, 8, 2, 128], F8, name=f"AL{p_}_{s}",
                           tag=f"AL{p_}_{s}") for s in range(2)]
                  for p_ in range(2)]

            # ---------------- phase 1: projections + rope -----------------
            # A-terms (hi*hi) run first across all groups so the wl/xl
            # streams have a whole A-pass to land; DMA queues are spread:
            # wh+xh(ch>0) on SP, wl+wo on Pool(SWDGE), xh(ch0)+xl+tables
            # on Act.
            # [kp][g][i][128] so the first DMA can be just pair0/g0 (256B)
            wh_sb = p1.tile([128, NKP, 6, 2, 128], F8, name="wh_sb",
                            tag="wh_sb")
            wl_sb = p1.tile([128, NKP, 6, 2, 128], F8, name="wl_sb",
                            tag="wl_sb")
            cos_sb = p1.tile([128, T], BF, name="cos_sb", tag="cos")
            sin_sb = p1.tile([128, T], BF, name="sin_sb", tag="sin")

            def wsl(w_sb, kp, g):
                # stationary [128, 2, 128] for pair kp, output group g
                return w_sb[:, kp, g, :, :]

            woh_sb = p2.tile([128, 2, 2, DIM], F8, name="woh_sb", tag="woh_sb")
            wol_sb = p2.tile([128, 2, 2, DIM], F8, name="wol_sb", tag="wol_sb")

            def scores_blk(s, qb, scpool, sctag, scbufs):
                tiles = QB_TILES[qb]
                n = len(tiles)
                qsl = slice(2 * qb * QB, 2 * (qb + 1) * QB)
                pts = []
                for p_ in range(2):
                    pt = p2.tile([128, MAXKT * 2 * QB], BF, name="pt",
                                 tag="pt", bufs=4)
                    pts.append(pt)
                    for gi in range(0, n, 2):
                        grp = tiles[gi:gi + 2]
                        w_ = len(grp) * 256
                        sc = scpool.tile([128, 512], F32, name="sc",
                                         tag=sctag, bufs=scbufs)
                        for i, (j, mi) in enumerate(grp):
                            nc.tensor.matmul(
                                sc[:, i * 256:(i + 1) * 256],
                                KTs[s][:, j * 128:(j + 1) * 128],
                                QTps[p_][s][:, qsl],
                                start=True, stop=True)
                        nc.scalar.activation(
                            pt[:, gi * 256:gi * 256 + w_], sc[:, 0:w_],
                            mybir.ActivationFunctionType.Exp, scale=SCALE)
                    for i, (j, mi) in enumerate(tiles):
                        if mi < 0:
                            continue
                        # SBUF-only, so Pool can own it; keeps DVE clear
                        # for the latency-critical normalize chain
                        nc.gpsimd.tensor_mul(
                            pt[:, i * 256:(i + 1) * 256],
                            pt[:, i * 256:(i + 1) * 256],
                            mask_sb[:, mi * 256:(mi + 1) * 256])
                return pts

            with tc.tile_pool(name="ps1", bufs=1, space="PSUM") as ps1:
                for ch in range(4):
                    s, loc = divmod(ch, 2)
                    csl = slice(ch * 512, (ch + 1) * 512)
                    xhs, xls = [], []
                    r0 = ch * 128
                    for b in range(4):   # blocks of 4 pairs
                        if ch == 0:
                            # wh streams on SP in block-matched pieces; the
                            # very first piece is just pair0/g0 so the PE
                            # starts ~0.5us earlier
                            if b == 0:
                                nc.sync.dma_start(wh_sb[:, 0:1, :, :, :],
                                                  wh[:, 0:1536])
                                nc.sync.dma_start(wh_sb[:, 1:4, :, :, :],
                                                  wh[:, 1536:4 * 1536])
                            else:
                                nc.sync.dma_start(
                                    wh_sb[:, b * 4:(b + 1) * 4, :, :, :],
                                    wh[:, b * 4 * 1536:(b + 1) * 4 * 1536])
                        th = p1.tile([128, 4, 2, 512], F8, name=f"xh{ch}_{b}",
                                     tag="xhs", bufs=4)
                        xhs.append(th)
                        c0 = b * 4096
                        xq = nc.scalar if ch == 0 else nc.sync
                        if ch == 0 and b == 0:
                            # first pair rides the Pool queue, ahead of wl
                            # (the Act queue starts late behind
                            # LoadActFuncSet; SP must stream wh unimpeded)
                            nc.gpsimd.dma_start(th[:, 0:1, :, :],
                                                xh[r0:r0 + 128, c0:c0 + 1024])
                            xq.dma_start(th[:, 1:4, :, :],
                                         xh[r0:r0 + 128, c0 + 1024:c0 + 4096])
                        else:
                            xq.dma_start(th[:],
                                         xh[r0:r0 + 128, c0:c0 + 4096])
                    for b in range(4):
                        # xl after all of xh: only needed from the C-pass on
                        tl_ = p1.tile([128, 4, 2, 512], F8, name=f"xl{ch}_{b}",
                                      tag="xls", bufs=4)
                        xls.append(tl_)
                        c0 = b * 4096
                        nc.scalar.dma_start(tl_[:],
                                            xl[r0:r0 + 128, c0:c0 + 4096])
                    if ch == 0:
                        # wl on the Pool SWDGE queue, in parallel with wh
                        nc.gpsimd.dma_start(wl_sb[:], wl[:])
                        # deferred small loads: needed only from RoPE time on
                        nc.scalar.dma_start(cos_sb[:], cosT[:])
                        nc.scalar.dma_start(sin_sb[:], sinT[:])
                        if _rep == 0:
                            nc.scalar.dma_start(aux_sb[:], aux[:])
                            nc.scalar.dma_start(vsc_sb[:], vsc[:])
                    if ch == 2:
                        # wo for phase 3, after the Pool queue drains wl
                        nc.gpsimd.dma_start(woh_sb[:], woh[:])
                        nc.gpsimd.dma_start(wol_sb[:], wol[:])
                    accs = [ps1.tile([128, 512], F32, name=f"acc{ch}_{g}",
                                     tag="acc", bufs=8) for g in range(6)]

                    def terms(kp):
                        b, kl = divmod(kp, 4)
                        mh = xhs[b][:, kl, :, :]
                        mlo = xls[b][:, kl, :, :]
                        return ((wh_sb, mh), (wl_sb, mh), (wh_sb, mlo))

                    for ti in (0, 1, 2):
                        for kp in range(NKP if ti == 0 else NKP - 4):
                            w_sb, mv = terms(kp)[ti]
                            for g in range(6):
                                nc.tensor.matmul(
                                    accs[g][:], wsl(w_sb, kp, g), mv,
                                    start=(kp == 0 and ti == 0),
                                    stop=False, perf_mode=DR)
                    # tail: finish each group's last 4 pairs (B/C terms) then
                    # emit its RoPE muls right away, so PSUM accs free
                    # progressively while later groups still accumulate.
                    # sin_sb holds [+sin; -sin] so rotated =
                    # acc*cos + swap_halves(acc*sin_sgn).
                    vtc = p1.tile([128, 512], BF, name="vtc", tag="vtc", bufs=1)
                    if ch == 3:
                        # first two blocks' scores live INSIDE the ps1 pool
                        # (acc-tag rotation slots), emitted before the tail:
                        # their exps head the Act queue, their matmuls keep
                        # the PE rolling across the ps1->psA close barrier
                        pre = [scores_blk(0, 0, ps1, "acc", 8),
                               scores_blk(0, 1, ps1, "acc", 8)]
                    for g in range(6):
                        for ti in (1, 2):
                            for kp in range(NKP - 4, NKP):
                                w_sb, mv = terms(kp)[ti]
                                nc.tensor.matmul(
                                    accs[g][:], wsl(w_sb, kp, g), mv,
                                    start=False,
                                    stop=(kp == NKP - 1 and ti == 2),
                                    perf_mode=DR)
                        if g < 4:
                            p_, e = divmod(g, 2)
                            dest = QTps[p_][s][:, loc * 1024 + e:
                                               (loc + 1) * 1024:2]
                        elif g == 4:
                            dest = KTs[s][:, loc * 512:(loc + 1) * 512]
                        else:
                            # V: descale (Sx*Sw) and apply ATT_SCALE in one
                            # per-partition scale-AP copy
                            nc.scalar.activation(
                                vtc[:], accs[5][:],
                                mybir.ActivationFunctionType.Copy,
                                scale=vscale)
                            break
                        # one Act copy PSUM->bf16 releases the acc quickly
                        # (GPSIMD can't read PSUM; DVE PSUM reads are 2x the
                        # cost of bf16 reads), then both rope muls run on
                        # DVE at the cheap 16-bit rate; per-g swap DMA (SP,
                        # clear of the Act queue) then the add on Pool
                        rt = p1.tile([128, 512], BF, name="rt", tag="rt",
                                     bufs=2)
                        bg = p1.tile([128, 512], BF, name="bg", tag="bg",
                                     bufs=2)
                        bs = p1.tile([128, 512], BF, name="bs", tag="bs",
                                     bufs=2)
                        # alternate engines so the serial per-tail chain
                        # (which gates acc release and the ph1->ph2 pool
                        # barrier) is split across Act and DVE
                        if g % 2 == 0:
                            nc.scalar.copy(rt[:], accs[g][:])
                        else:
                            nc.vector.tensor_copy(rt[:], accs[g][:])
                        nc.vector.tensor_mul(bg[:], rt[:], sin_sb[:, csl])
                        nc.vector.tensor_mul(dest, rt[:], cos_sb[:, csl])
                        nc.sync.dma_start(bs[0:64, :], bg[64:128, :])
                        nc.sync.dma_start(bs[64:128, :], bg[0:64, :])
                        # SBUF-only add on the (idle) Pool engine so the DVE
                        # queue is clear for attention's chain at ph1 end
                        nc.gpsimd.tensor_add(dest, dest, bs[:])
                    # V^T -> V via DMA XBAR transpose (no PE / PSUM use)
                    for q4 in range(4):
                        nc.sync.dma_start(Vc[ch][:, q4 * 128:(q4 + 1) * 128],
                                          vtc[:, q4 * 128:(q4 + 1) * 128],
                                          transpose=True)
            # ---------------- phase 2+3: attention + output proj ----------
            with tc.tile_pool(name="psA", bufs=1, space="PSUM") as psA:
                lrows = [[p2.tile([1, 2 * SEQ], BF, name=f"lrow{s}_{p_}",
                                  tag=f"lrow{s}_{p_}", bufs=1)
                          for p_ in range(2)] for s in range(NSEQ)]

                def pv_blk(s, qb, pts):
                    tiles = QB_TILES[qb]
                    n = len(tiles)
                    ovs = []
                    for p_ in range(2):
                        pt = pts[p_]
                        ov = psA.tile([128, 256], F32, name="ov",
                                      tag="ov", bufs=2)
                        ovs.append(ov)
                        for i, (j, _) in enumerate(tiles):
                            nc.tensor.matmul(
                                ov[:], Vc[s * 2 + j // 4][:, (j % 4) * 128:
                                                          (j % 4 + 1) * 128],
                                pt[:, i * 256:(i + 1) * 256],
                                start=(i == 0), stop=(i == n - 1))
                        # softmax denominator on Pool+DVE instead of PE:
                        # in-place cross-partition sum of the (now dead)
                        # probs, then row-accumulate across k-tiles.
                        nc.gpsimd.partition_all_reduce(
                            pt[:, 0:n * 256], pt[:, 0:n * 256],
                            channels=128, reduce_op=bass_isa.ReduceOp.add)
                        lsl = lrows[s][p_][0:1, qb * 256:(qb + 1) * 256]
                        nc.vector.tensor_copy(lsl, pt[0:1, 0:256])
                        for i in range(1, n):
                            nc.vector.tensor_add(
                                lsl, lsl, pt[0:1, i * 256:(i + 1) * 256])
                    return ovs

                def norm_blk(s, qb, ovs):
                    # denominators are final per block: normalize, then split
                    # into e4m3 hi/lo planes for the DoubleRow projection.
                    # Everything avoids the Act queue: exp latency is the PE
                    # critical path, so Act stays exp-only.
                    for p_ in range(2):
                        lsl = lrows[s][p_][0:1, qb * 256:(qb + 1) * 256]
                        with nc.allow_low_precision(reason="softmax denom scale"):
                            nc.vector.reciprocal(lsl, lsl)
                        lb = p2.tile([128, 256], BF, name="lb", tag="lb",
                                     bufs=2)
                        nc.gpsimd.partition_broadcast(lb[:], lsl)
                        at = p2.tile([128, 256], BF, name="at", tag="at",
                                     bufs=2)
                        nc.vector.tensor_mul(at[:], ovs[p_][:], lb[:])
                        for e in range(2):
                            hi = AH[p_][s][:, qb, e, :]
                            lo = AL[p_][s][:, qb, e, :]
                            nc.scalar.copy(hi, at[:, e::2])
                            nc.gpsimd.tensor_sub(lo, at[:, e::2], hi)

                def proj_tl(s, tl, last_call=False):
                    tb = s * 8 + tl
                    for sh in range(2):
                        stg = p2.tile([128, 2048], BF, name="stg",
                                      tag="stg", bufs=2)
                        last = last_call and sh == 1
                        for cc in range(4):
                            chn = sh * 4 + cc
                            oc = psA.tile([128, 512], F32, name="oc",
                                          tag="oc", bufs=4)
                            ti = 0
                            for p_ in range(2):
                                for stat, mov in ((AH, woh_sb), (AH, wol_sb),
                                                  (AL, woh_sb)):
                                    nc.tensor.matmul(
                                        oc[:],
                                        stat[p_][s][:, tl, :, :],
                                        mov[:, p_, :, chn * 512:
                                            (chn + 1) * 512],
                                        start=(ti == 0), stop=(ti == 5),
                                        perf_mode=DR)
                                    ti += 1
                            dsl = stg[:, cc * 512:(cc + 1) * 512]
                            if cc % 2 == 0:
                                nc.scalar.copy(dsl, oc[:])
                            else:
                                nc.vector.tensor_copy(dsl, oc[:])
                            if last:
                                # drain fast: per-chunk DMAs right after
                                # each copy, alternating queues
                                dq = nc.sync if cc % 2 == 0 else nc.gpsimd
                                dq.dma_start(
                                    outp[tb * 128:(tb + 1) * 128,
                                         chn * 512:(chn + 1) * 512], dsl)
                        if not last:
                            nc.sync.dma_start(
                                outp[tb * 128:(tb + 1) * 128,
                                     sh * 2048:(sh + 1) * 2048], stg[:])

                # per block: scores -> the (qb-2) projection (PE filler
                # while exp/mask run; two blocks of slack keeps the
                # multi-engine normalize/split chain off the PE critical
                # path) -> PV -> normalize+split
                # seq 1 ends with qb0: the final block's norm chain (1
                # k-tile) is the shortest, trimming the drain tail
                order = [(0, qb) for qb in range(SEQ // QB)] \
                    + [(1, qb) for qb in range(1, SEQ // QB)] + [(1, 0)]
                # scores run one block ahead so each block's exp/mask chain
                # has a whole iteration to complete before its PV; blocks 0
                # and 1 were pre-emitted inside the ps1 pool
                pts_next = None
                for i, (s, qb) in enumerate(order):
                    pts = pre[i] if i < 2 else pts_next
                    if 1 <= i < len(order) - 1:
                        pts_next = scores_blk(*order[i + 1], psA, "sc", 2)
                    if i >= 2:
                        proj_tl(*order[i - 2])
                    ovs = pv_blk(s, qb, pts)
                    norm_blk(s, qb, ovs)
                proj_tl(*order[-2])
                proj_tl(*order[-1], last_call=True)

    nc.compile()
    return nc


def _get_nc():
    if "nc" not in _NC_CACHE:
        _NC_CACHE["nc"] = _build_nc()
    return _NC_CACHE["nc"]


def _pow2_scale(absmax, target=224.0):
    return 2.0 ** np.floor(np.log2(target / max(absmax, 1e-30)))


def _split8(a, scale):
    """-> (hi, lo) e4m3 planes of a*scale (common power-of-2 scale)."""
    s = (a * scale).astype(np.float32)
    hi = s.astype(NPF8)
    lo = (s - hi.astype(np.float32)).astype(NPF8)
    return hi, lo


def _host_prep(x, cos, sin, wq, wk, wv, wo):
    perm = np.concatenate([np.arange(0, 128, 2), np.arange(1, 128, 2)])
    wq_p = wq.reshape(32, 128, DIM)[:, perm, :].reshape(32 * 128, DIM)
    wk_p = wk.reshape(8, 128, DIM)[:, perm, :].reshape(8 * 128, DIM)
    xT = np.ascontiguousarray(x.T)  # [DIM, T]

    sx = _pow2_scale(np.abs(x).max())
    sw = _pow2_scale(max(np.abs(wq).max(), np.abs(wk).max(), np.abs(wv).max()))
    swo = _pow2_scale(np.abs(wo).max())

    xh_, xl_ = _split8(xT, sx)
    # [DIM, T] -> [ch*128+p, kp*1024 + i*512 + c]
    def xlay(a):
        return np.ascontiguousarray(
            a.reshape(NKP, 2, 128, 4, 512).transpose(3, 2, 0, 1, 4)
            .reshape(4 * 128, NKP * 1024))
    xh_, xl_ = xlay(xh_), xlay(xl_)

    # rope tables absorb 1/(sx*sw)
    dsc = 1.0 / (sx * sw)
    cosT = (np.vstack([cos.T, cos.T]) * dsc).astype(NPBF)
    sinT = (np.vstack([sin.T, -sin.T]) * dsc).astype(NPBF)
    p = np.arange(128)[:, None]
    j = np.arange(QB)[None, :]
    masks = [(j >= p).astype(np.float32), (j < p).astype(np.float32)]
    aux = np.concatenate(
        [np.repeat(m, 2, axis=1) for m in masks]
        + [np.zeros((128, 2), np.float32)], axis=1).astype(NPBF)
    vsc = np.full((128, 1), ATT_SCALE * dsc, np.float32)

    in_maps = []
    for c in range(NCORE):
        wqkv = np.concatenate([
            wq_p[c * 512:(c + 1) * 512],
            wk_p[c * 128:(c + 1) * 128],
            wv[c * 128:(c + 1) * 128]], axis=0)  # [768, DIM]
        whh, wll = _split8(wqkv.T, sw)  # [DIM, 768]

        def wlay(a):
            # [p, kp, g, i, m]: group-major within a pair so the first DMA
            # can deliver just pair0/g0
            return np.ascontiguousarray(
                a.reshape(NKP, 2, 128, 6, 128).transpose(2, 0, 3, 1, 4)
                .reshape(128, NKP * 2 * WCOLS))
        wos = wo[:, c * 512:(c + 1) * 512].T  # [512 feat, DIM out]
        woh_, wol_ = _split8(wos, swo)

        def wolay(a):
            return np.ascontiguousarray(
                a.reshape(2, 2, 128, DIM).transpose(2, 0, 1, 3)
                .reshape(128, 2 * 2 * DIM))
        in_maps.append({
            "xh": xh_, "xl": xl_,
            "wh": wlay(whh), "wl": wlay(wll),
            "woh": wolay(woh_), "wol": wolay(wol_),
            "cosT": cosT, "sinT": sinT, "aux": aux, "vsc": vsc,
        })
    return in_maps, 1.0 / (ATT_SCALE * swo)


def kernel(x, cos, sin, wq, wk, wv, wo, n_seqs):
    x = np.asarray(x, dtype=np.float32)
    cos = np.asarray(cos, dtype=np.float32)
    sin = np.asarray(sin, dtype=np.float32)
    wq = np.asarray(wq, dtype=np.float32)
    wk = np.asarray(wk, dtype=np.float32)
    wv = np.asarray(wv, dtype=np.float32)
    wo = np.asarray(wo, dtype=np.float32)
    assert int(n_seqs) == NSEQ and x.shape == (T, DIM)

    nc = _get_nc()
    in_maps, out_dsc = _host_prep(x, cos, sin, wq, wk, wv, wo)
    res = run_bass_kernel_spmd(nc, in_maps, list(range(NCORE))).results
    out = np.zeros((T, DIM), dtype=np.float32)
    for c in range(NCORE):
        out += res[c]["outp"].astype(np.float32)
    return out * out_dsc


# revision 9
# speedup vs baseline: 1.0086x; 1.0020x over previous
"""Trainium2 Bass kernel for sparse (sliding-window, GQA, RoPE) attention.

Sharding: 8-way tensor-parallel over heads. Core c owns q-heads 4c..4c+3 and
kv-head c (wq/wk/wv column-parallel, wo row-parallel); each core produces a
full-shape partial output and the host sums the 8 partials (the all-reduce).

The two big projections (QKV and WO) run as fp8-e4m3 DoubleRow matmuls
with a 3-term residual decomposition
    W @ x ~= (W_hi + W_lo) @ x_hi + W_hi @ x_lo
where *_hi = e4m3(t), *_lo = e4m3(t - dequant(t_hi)) share one power-of-two
scale. Each DoubleRow instruction covers two 128-deep k-subtiles at 0.5
cyc/col, so the three terms cost 0.75 cyc/col/k-tile vs 1.0 for bf16 while
keeping ~bf16 precision (residuals are exact-scale floats; PSUM adds slots
1:1). x/w splits are host-prepped; the attention output's hi/lo split runs
on device (DVE mul -> bf16 tmp, Act cast-copy -> hi, Pool subtract -> lo).
Attention itself stays bf16 (softmax amplifies fp8 score noise; scores
contract over a single 128-wide head_dim so DoubleRow can't pair there).
Descale bookkeeping: QK via host-scaled cos/sin tables, V via one scale-AP
activation copy (attn sits at 16x device scale), final 1/(16*Swo) in the
host-side partial sum.

Schedule: phase 1 streams x hi/lo and runs A-terms (hi*hi) across all six
output groups first so the wl/xl DMA streams have a whole pass to land;
per-group tails emit RoPE (Act PSUM->bf16 copy frees the acc, DVE muls,
SP-queue partition-swap DMA, Pool add) progressively. Phase 2 runs per
128-token block: scores one block AHEAD (exp/mask latency hidden), the
(i-2) block's 48 DoubleRow output-projection matmuls as PE filler, then
PV and the normalize/split chain spread across DVE/Act/Pool. DMA queues:
wh+xh on SP, wl+wo+first-x on Pool SWDGE, xl+tables on Act, output on SP
with the final tile fanned across SP+Pool.
"""
import numpy as np
from contextlib import ExitStack

import ml_dtypes

import concourse.bass as bass
import concourse.bass_isa as bass_isa
from concourse import bacc
import concourse.mybir as mybir
import concourse.tile as tile
from concourse.bass_utils import run_bass_kernel_spmd

BF = mybir.dt.bfloat16
F8 = mybir.dt.float8e4
F32 = mybir.dt.float32
NPBF = ml_dtypes.bfloat16
NPF8 = ml_dtypes.float8_e4m3
DR = mybir.MatmulPerfMode.DoubleRow

NCORE = 8
T = 2048              # total tokens (2 seqs x 1024)
DIM = 4096
SEQ = 1024
NSEQ = 2
HD = 128              # head dim
NH = 4                # q heads per core
NKP = 16              # contraction k-tile PAIRS (32 tiles of 128)
QB = 128              # attention q-block width
SCALE = float(HD) ** -0.5
WCOLS = NH * HD + 2 * HD   # 768 projection output cols per core
ATT_SCALE = 16.0           # device-side scale carried by V/attn

# per-(seq-local qb) score k-tile lists: (seq-local k-tile index, mask id)
# masks: -1 none, 0: causal j>=p, 1: window j<p
QB_TILES = []
for _N in range(8):
    _lo = max(0, _N - 4)
    _tl = []
    for _j in range(_lo, _N + 1):
        _mi = -1
        if _j == _N - 4:
            _mi = 1
        if _j == _N:
            _mi = 0
        _tl.append((_j, _mi))
    QB_TILES.append(_tl)
MAXKT = 5

_NC_CACHE = {}


def _build_nc(reps=1, internal_io=False):
    nc = bacc.Bacc("TRN2", target_bir_lowering=False, debug=False,
                   num_devices=NCORE)
    if internal_io:
        # timing-only variant: big tensors live in device DRAM (no host
        # transfer per run); tiny dummy in/out keep the pjrt contract.
        def dram_in(name, shape, dt):
            return nc.dram_tensor(name, shape, dt).ap()
        dummy_in = nc.declare_dram_parameter("tin", [128, 128], F32, isOutput=False)
        dout = nc.declare_dram_parameter("tout", [128, 128], F32, isOutput=True)
        outp = nc.dram_tensor("outp_i", [T, DIM], BF).ap()
    else:
        def dram_in(name, shape, dt):
            return nc.declare_dram_parameter(name, shape, dt, isOutput=False)
        outp = nc.declare_dram_parameter("outp", [T, DIM], BF, isOutput=True)
    # host pre-arranged layouts (see _host_prep):
    #   xh/xl[ch*128+p, kp*1024 + i*512 + c] = e4m3(Sx * x[ch*512+c,
    #                                               (2kp+i)*128+p]) hi/lo
    #   wh/wl[p, kp*1536 + i*768 + j] = e4m3(Sw * wqkv[j, (2kp+i)*128+p])
    #   woh/wol[p, p_*8192 + e*4096 + o] = e4m3(Swo * wo[o,
    #                                          core*512 + (2p_+e)*128+p])
    xh = dram_in("xh", [4 * 128, NKP * 1024], F8)
    xl = dram_in("xl", [4 * 128, NKP * 1024], F8)
    wh = dram_in("wh", [128, NKP * 2 * WCOLS], F8)
    wl = dram_in("wl", [128, NKP * 2 * WCOLS], F8)
    woh = dram_in("woh", [128, 2 * 2 * DIM], F8)
    wol = dram_in("wol", [128, 2 * 2 * DIM], F8)
    cosT = dram_in("cosT", [128, T], BF)
    sinT = dram_in("sinT", [128, T], BF)
    aux = dram_in("aux", [128, 2 * 2 * QB + 2], BF)
    vsc = dram_in("vsc", [128, 1], F32)

    with tile.TileContext(nc) as tc, ExitStack() as top:
        persist = top.enter_context(tc.tile_pool(name="persist", bufs=1))
        if internal_io:
            # on the Pool SWDGE queue so the SP queue's first transfer is
            # the weights the PE is waiting on
            dtile = persist.tile([128, 32], F32, name="dtile", tag="dtile")
            nc.gpsimd.dma_start(dtile[:], dummy_in[:, 0:32])
            nc.gpsimd.dma_start(dout[:, 0:32], dtile[:])
            nc.gpsimd.dma_start(dout[:, 32:128], dummy_in[:, 32:128])

        aux_sb = persist.tile([128, 2 * 2 * QB + 2], BF, name="aux_sb", tag="aux")
        mask_sb = aux_sb[:, 0:2 * 2 * QB]
        vsc_sb = persist.tile([128, 1], F32, name="vsc_sb", tag="vsc")
        vscale = vsc_sb[:, 0:1]   # ATT_SCALE/(Sx*Sw), fp32 scale AP

        for _rep in range(reps):
         with ExitStack() as rep:
            pq = rep.enter_context(tc.tile_pool(name="pq", bufs=1))
            p1 = rep.enter_context(tc.tile_pool(name="p1", bufs=1))
            p2 = rep.enter_context(tc.tile_pool(name="p2", bufs=1))

            # cross-phase bf16 tiles
            QTps = [[pq.tile([128, 2 * SEQ], BF, name=f"QT{p_}_{s}",
                             tag=f"QT{p_}_{s}") for s in range(2)]
                    for p_ in range(2)]
            KTs = [pq.tile([128, SEQ], BF, name=f"KT{s}", tag=f"KT{s}")
                   for s in range(2)]
            Vc = [pq.tile([128, 512], BF, name=f"Vc{ch}", tag=f"Vc{ch}")
                  for ch in range(4)]
            # attention output, normalized, at ATT_SCALE, e4m3 hi/lo planes
            # laid out [128 feat, tl, head-in-pair, tok] for DoubleRow lhsT
            AH = [[pq.tile([128, 8, 2, 128], F8, name=f"AH{p_}_{s}",
                           tag=f"AH{p_}_{s}") for s in range(2)]
                  for p_ in range(2)]
            AL = [[pq.tile([128, 8, 2, 128], F8, name=f"AL{p_}_{s}",
                           tag=f"AL{p_}_{s}") for s in range(2)]
                  for p_ in range(2)]

            # ---------------- phase 1: projections + rope -----------------
            # A-terms (hi*hi) run first across all groups so the wl/xl
            # streams have a whole A-pass to land; DMA queues are spread:
            # wh+xh(ch>0) on SP, wl+wo on Pool(SWDGE), xh(ch0)+xl+tables
            # on Act.
            # [kp][g][i][128] so the first DMA can be just pair0/g0 (256B)
            wh_sb = p1.tile([128, NKP, 6, 2, 128], F8, name="wh_sb",
                            tag="wh_sb")
            wl_sb = p1.tile([128, NKP, 6, 2, 128], F8, name="wl_sb",
                            tag="wl_sb")
            cos_sb = p1.tile([128, T], BF, name="cos_sb", tag="cos")
            sin_sb = p1.tile([128, T], BF, name="sin_sb", tag="sin")

            def wsl(w_sb, kp, g):
                # stationary [128, 2, 128] for pair kp, output group g
                return w_sb[:, kp, g, :, :]

            woh_sb = p2.tile([128, 2, 2, DIM], F8, name="woh_sb", tag="woh_sb")
            wol_sb = p2.tile([128, 2, 2, DIM], F8, name="wol_sb", tag="wol_sb")

            def scores_blk(s, qb, scpool, sctag, scbufs):
                tiles = QB_TILES[qb]
                n = len(tiles)
                qsl = slice(2 * qb * QB, 2 * (qb + 1) * QB)
                pts = []
                for p_ in range(2):
                    pt = p2.tile([128, MAXKT * 2 * QB], BF, name="pt",
                                 tag="pt", bufs=4)
                    pts.append(pt)
                    for gi in range(0, n, 2):
                        grp = tiles[gi:gi + 2]
                        w_ = len(grp) * 256
                        sc = scpool.tile([128, 512], F32, name="sc",
                                         tag=sctag, bufs=scbufs)
                        for i, (j, mi) in enumerate(grp):
                            nc.tensor.matmul(
                                sc[:, i * 256:(i + 1) * 256],
                                KTs[s][:, j * 128:(j + 1) * 128],
                                QTps[p_][s][:, qsl],
                                start=True, stop=True)
                        nc.scalar.activation(
                            pt[:, gi * 256:gi * 256 + w_], sc[:, 0:w_],
                            mybir.ActivationFunctionType.Exp, scale=SCALE)
                    for i, (j, mi) in enumerate(tiles):
                        if mi < 0:
                            continue
                        # SBUF-only, so Pool can own it; keeps DVE clear
                        # for the latency-critical normalize chain
                        nc.gpsimd.tensor_mul(
                            pt[:, i * 256:(i + 1) * 256],
                            pt[:, i * 256:(i + 1) * 256],
                            mask_sb[:, mi * 256:(mi + 1) * 256])
                return pts

            with tc.tile_pool(name="ps1", bufs=1, space="PSUM") as ps1:
                for ch in range(4):
                    s, loc = divmod(ch, 2)
                    csl = slice(ch * 512, (ch + 1) * 512)
                    xhs, xls = [], []
                    r0 = ch * 128
                    for b in range(4):   # blocks of 4 pairs
                        if ch == 0:
                            # wh streams on SP in block-matched pieces; the
                            # very first piece is just pair0/g0 so the PE
                            # starts ~0.5us earlier
                            if b == 0:
                                nc.sync.dma_start(wh_sb[:, 0:1, :, :, :],
                                                  wh[:, 0:1536])
                                nc.sync.dma_start(wh_sb[:, 1:4, :, :, :],
                                                  wh[:, 1536:4 * 1536])
                            else:
                                nc.sync.dma_start(
                                    wh_sb[:, b * 4:(b + 1) * 4, :, :, :],
                                    wh[:, b * 4 * 1536:(b + 1) * 4 * 1536])
                        th = p1.tile([128, 4, 2, 512], F8, name=f"xh{ch}_{b}",
                                     tag="xhs", bufs=4)
                        xhs.append(th)
                        c0 = b * 4096
                        xq = nc.scalar if ch == 0 else nc.sync
                        if ch == 0 and b == 0:
                            # first pair rides the Pool queue, ahead of wl
                            # (the Act queue starts late behind
                            # LoadActFuncSet; SP must stream wh unimpeded)
                            nc.gpsimd.dma_start(th[:, 0:1, :, :],
                                                xh[r0:r0 + 128, c0:c0 + 1024])
                            xq.dma_start(th[:, 1:4, :, :],
                                         xh[r0:r0 + 128, c0 + 1024:c0 + 4096])
                        else:
                            xq.dma_start(th[:],
                                         xh[r0:r0 + 128, c0:c0 + 4096])
                    for b in range(4):
                        # xl after all of xh: only needed from the C-pass on
                        tl_ = p1.tile([128, 4, 2, 512], F8, name=f"xl{ch}_{b}",
                                      tag="xls", bufs=4)
                        xls.append(tl_)
                        c0 = b * 4096
                        nc.scalar.dma_start(tl_[:],
                                            xl[r0:r0 + 128, c0:c0 + 4096])
                    if ch == 0:
                        # wl on the Pool SWDGE queue, in parallel with wh
                        nc.gpsimd.dma_start(wl_sb[:], wl[:])
                        # deferred small loads: needed only from RoPE time on
                        nc.scalar.dma_start(cos_sb[:], cosT[:])
                        nc.scalar.dma_start(sin_sb[:], sinT[:])
                        if _rep == 0:
                            nc.scalar.dma_start(aux_sb[:], aux[:])
                            nc.scalar.dma_start(vsc_sb[:], vsc[:])
                    if ch == 2:
                        # wo for phase 3, after the Pool queue drains wl
                        nc.gpsimd.dma_start(woh_sb[:], woh[:])
                        nc.gpsimd.dma_start(wol_sb[:], wol[:])
                    accs = [ps1.tile([128, 512], F32, name=f"acc{ch}_{g}",
                                     tag="acc", bufs=8) for g in range(6)]

                    def terms(kp):
                        b, kl = divmod(kp, 4)
                        mh = xhs[b][:, kl, :, :]
                        mlo = xls[b][:, kl, :, :]
                        return ((wh_sb, mh), (wl_sb, mh), (wh_sb, mlo))

                    for ti in (0, 1, 2):
                        for kp in range(NKP if ti == 0 else NKP - 4):
                            w_sb, mv = terms(kp)[ti]
                            for g in range(6):
                                nc.tensor.matmul(
                                    accs[g][:], wsl(w_sb, kp, g), mv,
                                    start=(kp == 0 and ti == 0),
                                    stop=False, perf_mode=DR)
                    # tail: finish each group's last 4 pairs (B/C terms) then
                    # emit its RoPE muls right away, so PSUM accs free
                    # progressively while later groups still accumulate.
                    # sin_sb holds [+sin; -sin] so rotated =
                    # acc*cos + swap_halves(acc*sin_sgn).
                    vtc = p1.tile([128, 512], BF, name="vtc", tag="vtc", bufs=1)
                    if ch == 3:
                        # first two blocks' scores live INSIDE the ps1 pool
                        # (acc-tag rotation slots), emitted before the tail:
                        # their exps head the Act queue, their matmuls keep
                        # the PE rolling across the ps1->psA close barrier
                        pre = [scores_blk(0, 0, ps1, "acc", 8),
                               scores_blk(0, 1, ps1, "acc", 8)]
                    for g in range(6):
                        for ti in (1, 2):
                            for kp in range(NKP - 4, NKP):
                                w_sb, mv = terms(kp)[ti]
                                nc.tensor.matmul(
                                    accs[g][:], wsl(w_sb, kp, g), mv,
                                    start=False,
                                    stop=(kp == NKP - 1 and ti == 2),
                                    perf_mode=DR)
                        if g < 4:
                            p_, e = divmod(g, 2)
                            dest = QTps[p_][s][:, loc * 1024 + e:
                                               (loc + 1) * 1024:2]
                        elif g == 4:
                            dest = KTs[s][:, loc * 512:(loc + 1) * 512]
                        else:
                            # V: descale (Sx*Sw) and apply ATT_SCALE in one
                            # per-partition scale-AP copy
                            nc.scalar.activation(
                                vtc[:], accs[5][:],
                                mybir.ActivationFunctionType.Copy,
                                scale=vscale)
                            break
                        # one Act copy PSUM->bf16 releases the acc quickly
                        # (GPSIMD can't read PSUM; DVE PSUM reads are 2x the
                        # cost of bf16 reads), then both rope muls run on
                        # DVE at the cheap 16-bit rate; per-g swap DMA (SP,
                        # clear of the Act queue) then the add on Pool
                        rt = p1.tile([128, 512], BF, name="rt", tag="rt",
                                     bufs=2)
                        bg = p1.tile([128, 512], BF, name="bg", tag="bg",
                                     bufs=2)
                        bs = p1.tile([128, 512], BF, name="bs", tag="bs",
                                     bufs=2)
                        # alternate engines so the serial per-tail chain
                        # (which gates acc release and the ph1->ph2 pool
                        # barrier) is split across Act and DVE
                        if g % 2 == 0:
                            nc.scalar.copy(rt[:], accs[g][:])
                        else:
                            nc.vector.tensor_copy(rt[:], accs[g][:])
                        nc.vector.tensor_mul(bg[:], rt[:], sin_sb[:, csl])
                        nc.vector.tensor_mul(dest, rt[:], cos_sb[:, csl])
                        nc.sync.dma_start(bs[0:64, :], bg[64:128, :])
                        nc.sync.dma_start(bs[64:128, :], bg[0:64, :])
                        # SBUF-only add on the (idle) Pool engine so the DVE
                        # queue is clear for attention's chain at ph1 end
                        nc.gpsimd.tensor_add(dest, dest, bs[:])
                    # V^T -> V via DMA XBAR transpose (no PE / PSUM use)
                    for q4 in range(4):
                        nc.sync.dma_start(Vc[ch][:, q4 * 128:(q4 + 1) * 128],
                                          vtc[:, q4 * 128:(q4 + 1) * 128],
                                          transpose=True)
            # ---------------- phase 2+3: attention + output proj ----------
            with tc.tile_pool(name="psA", bufs=1, space="PSUM") as psA:
                lrows = [[p2.tile([1, 2 * SEQ], BF, name=f"lrow{s}_{p_}",
                                  tag=f"lrow{s}_{p_}", bufs=1)
                          for p_ in range(2)] for s in range(NSEQ)]

                def pv_blk(s, qb, pts):
                    tiles = QB_TILES[qb]
                    n = len(tiles)
                    ovs = []
                    for p_ in range(2):
                        pt = pts[p_]
                        ov = psA.tile([128, 256], F32, name="ov",
                                      tag="ov", bufs=2)
                        ovs.append(ov)
                        for i, (j, _) in enumerate(tiles):
                            nc.tensor.matmul(
                                ov[:], Vc[s * 2 + j // 4][:, (j % 4) * 128:
                                                          (j % 4 + 1) * 128],
                                pt[:, i * 256:(i + 1) * 256],
                                start=(i == 0), stop=(i == n - 1))
                        # softmax denominator on Pool+DVE instead of PE:
                        # in-place cross-partition sum of the (now dead)
                        # probs, then row-accumulate across k-tiles.
                        nc.gpsimd.partition_all_reduce(
                            pt[:, 0:n * 256], pt[:, 0:n * 256],
                            channels=128, reduce_op=bass_isa.ReduceOp.add)
                        lsl = lrows[s][p_][0:1, qb * 256:(qb + 1) * 256]
                        nc.vector.tensor_copy(lsl, pt[0:1, 0:256])
                        for i in range(1, n):
                            nc.vector.tensor_add(
                                lsl, lsl, pt[0:1, i * 256:(i + 1) * 256])
                    return ovs

                def norm_blk(s, qb, ovs):
                    # denominators are final per block: normalize, then split
                    # into e4m3 hi/lo planes for the DoubleRow projection.
                    # Everything avoids the Act queue: exp latency is the PE
                    # critical path, so Act stays exp-only.
                    for p_ in range(2):
                        lsl = lrows[s][p_][0:1, qb * 256:(qb + 1) * 256]
                        with nc.allow_low_precision(reason="softmax denom scale"):
                            nc.vector.reciprocal(lsl, lsl)
                        lb = p2.tile([128, 256], BF, name="lb", tag="lb",
                                     bufs=2)
                        nc.gpsimd.partition_broadcast(lb[:], lsl)
                        at = p2.tile([128, 256], BF, name="at", tag="at",
                                     bufs=2)
                        nc.vector.tensor_mul(at[:], ovs[p_][:], lb[:])
                        for e in range(2):
                            hi = AH[p_][s][:, qb, e, :]
                            lo = AL[p_][s][:, qb, e, :]
                            nc.scalar.copy(hi, at[:, e::2])
                            nc.gpsimd.tensor_sub(lo, at[:, e::2], hi)

                def proj_tl(s, tl, last_call=False, dmaq=None):
                    dmaq = dmaq or nc.sync
                    tb = s * 8 + tl
                    for sh in range(2):
                        stg = p2.tile([128, 2048], BF, name="stg",
                                      tag="stg", bufs=2)
                        last = last_call and sh == 1
                        for cc in range(4):
                            chn = sh * 4 + cc
                            oc = psA.tile([128, 512], F32, name="oc",
                                          tag="oc", bufs=4)
                            ti = 0
                            for p_ in range(2):
                                for stat, mov in ((AH, woh_sb), (AH, wol_sb),
                                                  (AL, woh_sb)):
                                    nc.tensor.matmul(
                                        oc[:],
                                        stat[p_][s][:, tl, :, :],
                                        mov[:, p_, :, chn * 512:
                                            (chn + 1) * 512],
                                        start=(ti == 0), stop=(ti == 5),
                                        perf_mode=DR)
                                    ti += 1
                            dsl = stg[:, cc * 512:(cc + 1) * 512]
                            if last and cc == 3:
                                # final chunk: split copy+DMA into halves on
                                # both engines and both queues so the drain
                                # chain (last copy -> last DMA completion)
                                # shortens
                                nc.scalar.copy(dsl[:, 0:256], oc[:, 0:256])
                                nc.gpsimd.dma_start(
                                    outp[tb * 128:(tb + 1) * 128,
                                         chn * 512:chn * 512 + 256],
                                    dsl[:, 0:256])
                                nc.vector.tensor_copy(dsl[:, 256:512],
                                                      oc[:, 256:512])
                                nc.sync.dma_start(
                                    outp[tb * 128:(tb + 1) * 128,
                                         chn * 512 + 256:(chn + 1) * 512],
                                    dsl[:, 256:512])
                                continue
                            if cc % 2 == 0:
                                nc.scalar.copy(dsl, oc[:])
                            else:
                                nc.vector.tensor_copy(dsl, oc[:])
                            if last:
                                # drain fast: per-chunk DMAs right after
                                # each copy, alternating queues; SP (fast
                                # HWDGE) takes the later chunks
                                dq = nc.gpsimd if cc % 2 == 0 else nc.sync
                                dq.dma_start(
                                    outp[tb * 128:(tb + 1) * 128,
                                         chn * 512:(chn + 1) * 512], dsl)
                        if not last:
                            dmaq.dma_start(
                                outp[tb * 128:(tb + 1) * 128,
                                     sh * 2048:(sh + 1) * 2048], stg[:])

                # per block: scores -> the (qb-2) projection (PE filler
                # while exp/mask run; two blocks of slack keeps the
                # multi-engine normalize/split chain off the PE critical
                # path) -> PV -> normalize+split
                # seq 1 ends with qb0: the final block's norm chain (1
                # k-tile) is the shortest, trimming the drain tail
                order = [(0, qb) for qb in range(SEQ // QB)] \
                    + [(1, qb) for qb in range(1, SEQ // QB)] + [(1, 0)]
                # scores run one block ahead so each block's exp/mask chain
                # has a whole iteration to complete before its PV; blocks 0
                # and 1 were pre-emitted inside the ps1 pool
                pts_next = None
                for i, (s, qb) in enumerate(order):
                    pts = pre[i] if i < 2 else pts_next
                    if 1 <= i < len(order) - 1:
                        pts_next = scores_blk(*order[i + 1], psA, "sc", 2)
                    if i >= 2:
                        proj_tl(*order[i - 2])
                    ovs = pv_blk(s, qb, pts)
                    norm_blk(s, qb, ovs)
                # the penultimate projection's big stg DMAs ride the Pool
                # queue so SP is clear for the final fan-out + drain
                proj_tl(*order[-2], dmaq=nc.gpsimd)
                proj_tl(*order[-1], last_call=True)

    nc.compile()
    return nc


def _get_nc():
    if "nc" not in _NC_CACHE:
        _NC_CACHE["nc"] = _build_nc()
    return _NC_CACHE["nc"]


def _pow2_scale(absmax, target=224.0):
    return 2.0 ** np.floor(np.log2(target / max(absmax, 1e-30)))


def _split8(a, scale):
    """-> (hi, lo) e4m3 planes of a*scale (common power-of-2 scale)."""
    s = (a * scale).astype(np.float32)
    hi = s.astype(NPF8)
    lo = (s - hi.astype(np.float32)).astype(NPF8)
    return hi, lo


def _host_prep(x, cos, sin, wq, wk, wv, wo):
    perm = np.concatenate([np.arange(0, 128, 2), np.arange(1, 128, 2)])
    wq_p = wq.reshape(32, 128, DIM)[:, perm, :].reshape(32 * 128, DIM)
    wk_p = wk.reshape(8, 128, DIM)[:, perm, :].reshape(8 * 128, DIM)
    xT = np.ascontiguousarray(x.T)  # [DIM, T]

    sx = _pow2_scale(np.abs(x).max())
    sw = _pow2_scale(max(np.abs(wq).max(), np.abs(wk).max(), np.abs(wv).max()))
    swo = _pow2_scale(np.abs(wo).max())

    xh_, xl_ = _split8(xT, sx)
    # [DIM, T] -> [ch*128+p, kp*1024 + i*512 + c]
    def xlay(a):
        return np.ascontiguousarray(
            a.reshape(NKP, 2, 128, 4, 512).transpose(3, 2, 0, 1, 4)
            .reshape(4 * 128, NKP * 1024))
    xh_, xl_ = xlay(xh_), xlay(xl_)

    # rope tables absorb 1/(sx*sw)
    dsc = 1.0 / (sx * sw)
    cosT = (np.vstack([cos.T, cos.T]) * dsc).astype(NPBF)
    sinT = (np.vstack([sin.T, -sin.T]) * dsc).astype(NPBF)
    p = np.arange(128)[:, None]
    j = np.arange(QB)[None, :]
    masks = [(j >= p).astype(np.float32), (j < p).astype(np.float32)]
    aux = np.concatenate(
        [np.repeat(m, 2, axis=1) for m in masks]
        + [np.zeros((128, 2), np.float32)], axis=1).astype(NPBF)
    vsc = np.full((128, 1), ATT_SCALE * dsc, np.float32)

    in_maps = []
    for c in range(NCORE):
        wqkv = np.concatenate([
            wq_p[c * 512:(c + 1) * 512],
            wk_p[c * 128:(c + 1) * 128],
            wv[c * 128:(c + 1) * 128]], axis=0)  # [768, DIM]
        whh, wll = _split8(wqkv.T, sw)  # [DIM, 768]

        def wlay(a):
            # [p, kp, g, i, m]: group-major within a pair so the first DMA
            # can deliver just pair0/g0
            return np.ascontiguousarray(
                a.reshape(NKP, 2, 128, 6, 128).transpose(2, 0, 3, 1, 4)
                .reshape(128, NKP * 2 * WCOLS))
        wos = wo[:, c * 512:(c + 1) * 512].T  # [512 feat, DIM out]
        woh_, wol_ = _split8(wos, swo)

        def wolay(a):
            return np.ascontiguousarray(
                a.reshape(2, 2, 128, DIM).transpose(2, 0, 1, 3)
                .reshape(128, 2 * 2 * DIM))
        in_maps.append({
            "xh": xh_, "xl": xl_,
            "wh": wlay(whh), "wl": wlay(wll),
            "woh": wolay(woh_), "wol": wolay(wol_),
            "cosT": cosT, "sinT": sinT, "aux": aux, "vsc": vsc,
        })
    return in_maps, 1.0 / (ATT_SCALE * swo)


def kernel(x, cos, sin, wq, wk, wv, wo, n_seqs):
    x = np.asarray(x, dtype=np.float32)
    cos = np.asarray(cos, dtype=np.float32)
    sin = np.asarray(sin, dtype=np.float32)
    wq = np.asarray(wq, dtype=np.float32)
    wk = np.asarray(wk, dtype=np.float32)
    wv = np.asarray(wv, dtype=np.float32)
    wo = np.asarray(wo, dtype=np.float32)
    assert int(n_seqs) == NSEQ and x.shape == (T, DIM)

    nc = _get_nc()
    in_maps, out_dsc = _host_prep(x, cos, sin, wq, wk, wv, wo)
    res = run_bass_kernel_spmd(nc, in_maps, list(range(NCORE))).results
    out = np.zeros((T, DIM), dtype=np.float32)
    for c in range(NCORE):
        out += res[c]["outp"].astype(np.float32)
    return out * out_dsc


# revision 10
# speedup vs baseline: 1.0125x; 1.0039x over previous
"""Trainium2 Bass kernel for sparse (sliding-window, GQA, RoPE) attention.

Sharding: 8-way tensor-parallel over heads. Core c owns q-heads 4c..4c+3 and
kv-head c (wq/wk/wv column-parallel, wo row-parallel); each core produces a
full-shape partial output and the host sums the 8 partials (the all-reduce).

The two big projections (QKV and WO) run as fp8-e4m3 DoubleRow matmuls
with a 3-term residual decomposition
    W @ x ~= (W_hi + W_lo) @ x_hi + W_hi @ x_lo
where *_hi = e4m3(t), *_lo = e4m3(t - dequant(t_hi)) share one power-of-two
scale. Each DoubleRow instruction covers two 128-deep k-subtiles at 0.5
cyc/col, so the three terms cost 0.75 cyc/col/k-tile vs 1.0 for bf16 while
keeping ~bf16 precision (residuals are exact-scale floats; PSUM adds slots
1:1). x/w splits are host-prepped; the attention output's hi/lo split runs
on device (DVE mul -> bf16 tmp, Act cast-copy -> hi, Pool subtract -> lo).
Attention itself stays bf16 (softmax amplifies fp8 score noise; scores
contract over a single 128-wide head_dim so DoubleRow can't pair there).
Descale bookkeeping: QK via host-scaled cos/sin tables, V via one scale-AP
activation copy (attn sits at 16x device scale), final 1/(16*Swo) in the
host-side partial sum.

Schedule: phase 1 streams x hi/lo and runs A-terms (hi*hi) across all six
output groups first so the wl/xl DMA streams have a whole pass to land;
per-group tails emit RoPE (Act PSUM->bf16 copy frees the acc, DVE muls,
SP-queue partition-swap DMA, Pool add) progressively. Phase 2 runs per
128-token block: scores one block AHEAD (exp/mask latency hidden), the
(i-2) block's 48 DoubleRow output-projection matmuls as PE filler, then
PV and the normalize/split chain spread across DVE/Act/Pool. DMA queues:
wh+xh on SP, wl+wo+first-x on Pool SWDGE, xl+tables on Act, output on SP
with the final tile fanned across SP+Pool.
"""
import numpy as np
from contextlib import ExitStack

import ml_dtypes

import concourse.bass as bass
import concourse.bass_isa as bass_isa
from concourse import bacc
import concourse.mybir as mybir
import concourse.tile as tile
from concourse.bass_utils import run_bass_kernel_spmd

BF = mybir.dt.bfloat16
F8 = mybir.dt.float8e4
F32 = mybir.dt.float32
NPBF = ml_dtypes.bfloat16
NPF8 = ml_dtypes.float8_e4m3
DR = mybir.MatmulPerfMode.DoubleRow

NCORE = 8
T = 2048              # total tokens (2 seqs x 1024)
DIM = 4096
SEQ = 1024
NSEQ = 2
HD = 128              # head dim
NH = 4                # q heads per core
NKP = 16              # contraction k-tile PAIRS (32 tiles of 128)
QB = 128              # attention q-block width
SCALE = float(HD) ** -0.5
WCOLS = NH * HD + 2 * HD   # 768 projection output cols per core
ATT_SCALE = 16.0           # device-side scale carried by V/attn

# per-(seq-local qb) score k-tile lists: (seq-local k-tile index, mask id)
# masks: -1 none, 0: causal j>=p, 1: window j<p
QB_TILES = []
for _N in range(8):
    _lo = max(0, _N - 4)
    _tl = []
    for _j in range(_lo, _N + 1):
        _mi = -1
        if _j == _N - 4:
            _mi = 1
        if _j == _N:
            _mi = 0
        _tl.append((_j, _mi))
    QB_TILES.append(_tl)
MAXKT = 5

_NC_CACHE = {}


def _build_nc(reps=1, internal_io=False):
    nc = bacc.Bacc("TRN2", target_bir_lowering=False, debug=False,
                   num_devices=NCORE)
    if internal_io:
        # timing-only variant: big tensors live in device DRAM (no host
        # transfer per run); tiny dummy in/out keep the pjrt contract.
        def dram_in(name, shape, dt):
            return nc.dram_tensor(name, shape, dt).ap()
        dummy_in = nc.declare_dram_parameter("tin", [128, 128], F32, isOutput=False)
        dout = nc.declare_dram_parameter("tout", [128, 128], F32, isOutput=True)
        outp = nc.dram_tensor("outp_i", [T, DIM], BF).ap()
    else:
        def dram_in(name, shape, dt):
            return nc.declare_dram_parameter(name, shape, dt, isOutput=False)
        outp = nc.declare_dram_parameter("outp", [T, DIM], BF, isOutput=True)
    # host pre-arranged layouts (see _host_prep):
    #   xh/xl[ch*128+p, kp*1024 + i*512 + c] = e4m3(Sx * x[ch*512+c,
    #                                               (2kp+i)*128+p]) hi/lo
    #   wh/wl[p, kp*1536 + i*768 + j] = e4m3(Sw * wqkv[j, (2kp+i)*128+p])
    #   woh/wol[p, p_*8192 + e*4096 + o] = e4m3(Swo * wo[o,
    #                                          core*512 + (2p_+e)*128+p])
    xh = dram_in("xh", [4 * 128, NKP * 1024], F8)
    xl = dram_in("xl", [4 * 128, NKP * 1024], F8)
    wh = dram_in("wh", [128, NKP * 2 * WCOLS], F8)
    wl = dram_in("wl", [128, NKP * 2 * WCOLS], F8)
    woh = dram_in("woh", [128, 2 * 2 * DIM], F8)
    wol = dram_in("wol", [128, 2 * 2 * DIM], F8)
    cosT = dram_in("cosT", [128, T], BF)
    sinT = dram_in("sinT", [128, T], BF)
    aux = dram_in("aux", [128, 2 * 2 * QB + 2], BF)
    vsc = dram_in("vsc", [128, 1], F32)

    with tile.TileContext(nc) as tc, ExitStack() as top:
        persist = top.enter_context(tc.tile_pool(name="persist", bufs=1))
        if internal_io:
            # on the Pool SWDGE queue so the SP queue's first transfer is
            # the weights the PE is waiting on
            dtile = persist.tile([128, 32], F32, name="dtile", tag="dtile")
            nc.gpsimd.dma_start(dtile[:], dummy_in[:, 0:32])
            nc.gpsimd.dma_start(dout[:, 0:32], dtile[:])
            nc.gpsimd.dma_start(dout[:, 32:128], dummy_in[:, 32:128])

        aux_sb = persist.tile([128, 2 * 2 * QB + 2], BF, name="aux_sb", tag="aux")
        mask_sb = aux_sb[:, 0:2 * 2 * QB]
        vsc_sb = persist.tile([128, 1], F32, name="vsc_sb", tag="vsc")
        vscale = vsc_sb[:, 0:1]   # ATT_SCALE/(Sx*Sw), fp32 scale AP

        for _rep in range(reps):
         with ExitStack() as rep:
            pq = rep.enter_context(tc.tile_pool(name="pq", bufs=1))
            p1 = rep.enter_context(tc.tile_pool(name="p1", bufs=1))
            p2 = rep.enter_context(tc.tile_pool(name="p2", bufs=1))

            # cross-phase bf16 tiles
            QTps = [[pq.tile([128, 2 * SEQ], BF, name=f"QT{p_}_{s}",
                             tag=f"QT{p_}_{s}") for s in range(2)]
                    for p_ in range(2)]
            KTs = [pq.tile([128, SEQ], BF, name=f"KT{s}", tag=f"KT{s}")
                   for s in range(2)]
            Vc = [pq.tile([128, 512], BF, name=f"Vc{ch}", tag=f"Vc{ch}")
                  for ch in range(4)]
            # attention output, normalized, at ATT_SCALE, e4m3 hi/lo planes
            # laid out [128 feat, tl, head-in-pair, tok] for DoubleRow lhsT
            AH = [[pq.tile([128, 8, 2, 128], F8, name=f"AH{p_}_{s}",
                           tag=f"AH{p_}_{s}") for s in range(2)]
                  for p_ in range(2)]
            AL = [[pq.tile([128, 8, 2, 128], F8, name=f"AL{p_}_{s}",
                           tag=f"AL{p_}_{s}") for s in range(2)]
                  for p_ in range(2)]

            # ---------------- phase 1: projections + rope -----------------
            # A-terms (hi*hi) run first across all groups so the wl/xl
            # streams have a whole A-pass to land; DMA queues are spread:
            # wh+xh(ch>0) on SP, wl+wo on Pool(SWDGE), xh(ch0)+xl+tables
            # on Act.
            # [kp][g][i][128] so the first DMA can be just pair0/g0 (256B)
            wh_sb = p1.tile([128, NKP, 6, 2, 128], F8, name="wh_sb",
                            tag="wh_sb")
            wl_sb = p1.tile([128, NKP, 6, 2, 128], F8, name="wl_sb",
                            tag="wl_sb")
            cos_sb = p1.tile([128, T], BF, name="cos_sb", tag="cos")
            sin_sb = p1.tile([128, T], BF, name="sin_sb", tag="sin")

            def wsl(w_sb, kp, g):
                # stationary [128, 2, 128] for pair kp, output group g
                return w_sb[:, kp, g, :, :]

            woh_sb = p2.tile([128, 2, 2, DIM], F8, name="woh_sb", tag="woh_sb")
            wol_sb = p2.tile([128, 2, 2, DIM], F8, name="wol_sb", tag="wol_sb")

            def scores_blk(s, qb, scpool, sctag, scbufs):
                tiles = QB_TILES[qb]
                n = len(tiles)
                qsl = slice(2 * qb * QB, 2 * (qb + 1) * QB)
                pts = []
                for p_ in range(2):
                    pt = p2.tile([128, MAXKT * 2 * QB], BF, name="pt",
                                 tag="pt", bufs=4)
                    pts.append(pt)
                    for gi in range(0, n, 2):
                        grp = tiles[gi:gi + 2]
                        w_ = len(grp) * 256
                        sc = scpool.tile([128, 512], F32, name="sc",
                                         tag=sctag, bufs=scbufs)
                        for i, (j, mi) in enumerate(grp):
                            nc.tensor.matmul(
                                sc[:, i * 256:(i + 1) * 256],
                                KTs[s][:, j * 128:(j + 1) * 128],
                                QTps[p_][s][:, qsl],
                                start=True, stop=True)
                        nc.scalar.activation(
                            pt[:, gi * 256:gi * 256 + w_], sc[:, 0:w_],
                            mybir.ActivationFunctionType.Exp, scale=SCALE)
                    for i, (j, mi) in enumerate(tiles):
                        if mi < 0:
                            continue
                        # SBUF-only, so Pool can own it; keeps DVE clear
                        # for the latency-critical normalize chain
                        nc.gpsimd.tensor_mul(
                            pt[:, i * 256:(i + 1) * 256],
                            pt[:, i * 256:(i + 1) * 256],
                            mask_sb[:, mi * 256:(mi + 1) * 256])
                return pts

            with tc.tile_pool(name="ps1", bufs=1, space="PSUM") as ps1:
                for ch in range(4):
                    s, loc = divmod(ch, 2)
                    csl = slice(ch * 512, (ch + 1) * 512)
                    xhs, xls = [], []
                    r0 = ch * 128
                    for b in range(4):   # blocks of 4 pairs
                        if ch == 0:
                            # wh streams on SP in block-matched pieces; the
                            # very first piece is just pair0/g0 so the PE
                            # starts ~0.5us earlier
                            if b == 0:
                                nc.sync.dma_start(wh_sb[:, 0:1, :, :, :],
                                                  wh[:, 0:1536])
                                nc.sync.dma_start(wh_sb[:, 1:4, :, :, :],
                                                  wh[:, 1536:4 * 1536])
                            else:
                                nc.sync.dma_start(
                                    wh_sb[:, b * 4:(b + 1) * 4, :, :, :],
                                    wh[:, b * 4 * 1536:(b + 1) * 4 * 1536])
                        th = p1.tile([128, 4, 2, 512], F8, name=f"xh{ch}_{b}",
                                     tag="xhs", bufs=4)
                        xhs.append(th)
                        c0 = b * 4096
                        xq = nc.scalar if ch == 0 else nc.sync
                        if ch == 0 and b == 0:
                            # first pair rides the Pool queue, ahead of wl
                            # (the Act queue starts late behind
                            # LoadActFuncSet; SP must stream wh unimpeded)
                            nc.gpsimd.dma_start(th[:, 0:1, :, :],
                                                xh[r0:r0 + 128, c0:c0 + 1024])
                            xq.dma_start(th[:, 1:4, :, :],
                                         xh[r0:r0 + 128, c0 + 1024:c0 + 4096])
                        else:
                            xq.dma_start(th[:],
                                         xh[r0:r0 + 128, c0:c0 + 4096])
                    for b in range(4):
                        # xl after all of xh: only needed from the C-pass on
                        tl_ = p1.tile([128, 4, 2, 512], F8, name=f"xl{ch}_{b}",
                                      tag="xls", bufs=4)
                        xls.append(tl_)
                        c0 = b * 4096
                        nc.scalar.dma_start(tl_[:],
                                            xl[r0:r0 + 128, c0:c0 + 4096])
                    if ch == 0:
                        # wl on the Pool SWDGE queue, in parallel with wh
                        nc.gpsimd.dma_start(wl_sb[:], wl[:])
                        # deferred small loads: needed only from RoPE time on
                        nc.scalar.dma_start(cos_sb[:], cosT[:])
                        nc.scalar.dma_start(sin_sb[:], sinT[:])
                        if _rep == 0:
                            nc.scalar.dma_start(aux_sb[:], aux[:])
                            nc.scalar.dma_start(vsc_sb[:], vsc[:])
                    if ch == 2:
                        # wo for phase 3, after the Pool queue drains wl
                        nc.gpsimd.dma_start(woh_sb[:], woh[:])
                        nc.gpsimd.dma_start(wol_sb[:], wol[:])
                    accs = [ps1.tile([128, 512], F32, name=f"acc{ch}_{g}",
                                     tag="acc", bufs=8) for g in range(6)]

                    def terms(kp):
                        b, kl = divmod(kp, 4)
                        mh = xhs[b][:, kl, :, :]
                        mlo = xls[b][:, kl, :, :]
                        return ((wh_sb, mh), (wl_sb, mh), (wh_sb, mlo))

                    for ti in (0, 1, 2):
                        for kp in range(NKP if ti == 0 else NKP - 4):
                            w_sb, mv = terms(kp)[ti]
                            for g in range(6):
                                nc.tensor.matmul(
                                    accs[g][:], wsl(w_sb, kp, g), mv,
                                    start=(kp == 0 and ti == 0),
                                    stop=False, perf_mode=DR)
                    # tail: finish each group's last 4 pairs (B/C terms) then
                    # emit its RoPE muls right away, so PSUM accs free
                    # progressively while later groups still accumulate.
                    # sin_sb holds [+sin; -sin] so rotated =
                    # acc*cos + swap_halves(acc*sin_sgn).
                    vtc = p1.tile([128, 512], BF, name="vtc", tag="vtc", bufs=1)
                    if ch == 3:
                        # first two blocks' scores live INSIDE the ps1 pool
                        # (acc-tag rotation slots), emitted before the tail:
                        # their exps head the Act queue, their matmuls keep
                        # the PE rolling across the ps1->psA close barrier
                        pre = [scores_blk(0, 0, ps1, "acc", 8),
                               scores_blk(0, 1, ps1, "acc", 8)]
                    for g in range(6):
                        for ti in (1, 2):
                            for kp in range(NKP - 4, NKP):
                                w_sb, mv = terms(kp)[ti]
                                nc.tensor.matmul(
                                    accs[g][:], wsl(w_sb, kp, g), mv,
                                    start=False,
                                    stop=(kp == NKP - 1 and ti == 2),
                                    perf_mode=DR)
                        if g < 4:
                            p_, e = divmod(g, 2)
                            dest = QTps[p_][s][:, loc * 1024 + e:
                                               (loc + 1) * 1024:2]
                        elif g == 4:
                            dest = KTs[s][:, loc * 512:(loc + 1) * 512]
                        else:
                            # V: descale (Sx*Sw) and apply ATT_SCALE in one
                            # per-partition scale-AP copy
                            nc.scalar.activation(
                                vtc[:], accs[5][:],
                                mybir.ActivationFunctionType.Copy,
                                scale=vscale)
                            break
                        # one Act copy PSUM->bf16 releases the acc quickly
                        # (GPSIMD can't read PSUM; DVE PSUM reads are 2x the
                        # cost of bf16 reads), then both rope muls run on
                        # DVE at the cheap 16-bit rate; per-g swap DMA (SP,
                        # clear of the Act queue) then the add on Pool
                        rt = p1.tile([128, 512], BF, name="rt", tag="rt",
                                     bufs=2)
                        bg = p1.tile([128, 512], BF, name="bg", tag="bg",
                                     bufs=2)
                        bs = p1.tile([128, 512], BF, name="bs", tag="bs",
                                     bufs=2)
                        # alternate engines so the serial per-tail chain
                        # (which gates acc release and the ph1->ph2 pool
                        # barrier) is split across Act and DVE
                        if g % 2 == 0:
                            nc.scalar.copy(rt[:], accs[g][:])
                        else:
                            nc.vector.tensor_copy(rt[:], accs[g][:])
                        nc.vector.tensor_mul(bg[:], rt[:], sin_sb[:, csl])
                        nc.vector.tensor_mul(dest, rt[:], cos_sb[:, csl])
                        nc.sync.dma_start(bs[0:64, :], bg[64:128, :])
                        nc.sync.dma_start(bs[64:128, :], bg[0:64, :])
                        # SBUF-only add on the (idle) Pool engine so the DVE
                        # queue is clear for attention's chain at ph1 end
                        nc.gpsimd.tensor_add(dest, dest, bs[:])
                    # V^T -> V via DMA XBAR transpose (no PE / PSUM use)
                    for q4 in range(4):
                        nc.sync.dma_start(Vc[ch][:, q4 * 128:(q4 + 1) * 128],
                                          vtc[:, q4 * 128:(q4 + 1) * 128],
                                          transpose=True)
            # ---------------- phase 2+3: attention + output proj ----------
            with tc.tile_pool(name="psA", bufs=1, space="PSUM") as psA:
                lrows = [[p2.tile([1, 2 * SEQ], BF, name=f"lrow{s}_{p_}",
                                  tag=f"lrow{s}_{p_}", bufs=1)
                          for p_ in range(2)] for s in range(NSEQ)]

                def pv_blk(s, qb, pts):
                    tiles = QB_TILES[qb]
                    n = len(tiles)
                    # both p_ halves share one PSUM bank (disjoint regions),
                    # doubling the effective ring depth so PV never waits
                    # the previous block's normalize chain
                    ov_t = psA.tile([128, 512], F32, name="ov",
                                    tag="ov", bufs=2)
                    ovs = []
                    for p_ in range(2):
                        pt = pts[p_]
                        ov = ov_t[:, p_ * 256:(p_ + 1) * 256]
                        ovs.append(ov)
                        for i, (j, _) in enumerate(tiles):
                            nc.tensor.matmul(
                                ov[:], Vc[s * 2 + j // 4][:, (j % 4) * 128:
                                                          (j % 4 + 1) * 128],
                                pt[:, i * 256:(i + 1) * 256],
                                start=(i == 0), stop=(i == n - 1))
                        # softmax denominator on Pool+DVE instead of PE:
                        # in-place cross-partition sum of the (now dead)
                        # probs, then row-accumulate across k-tiles.
                        nc.gpsimd.partition_all_reduce(
                            pt[:, 0:n * 256], pt[:, 0:n * 256],
                            channels=128, reduce_op=bass_isa.ReduceOp.add)
                        lsl = lrows[s][p_][0:1, qb * 256:(qb + 1) * 256]
                        nc.vector.tensor_copy(lsl, pt[0:1, 0:256])
                        for i in range(1, n):
                            nc.vector.tensor_add(
                                lsl, lsl, pt[0:1, i * 256:(i + 1) * 256])
                    return ovs

                def norm_blk(s, qb, ovs):
                    # denominators are final per block: normalize, then split
                    # into e4m3 hi/lo planes for the DoubleRow projection.
                    # Everything avoids the Act queue: exp latency is the PE
                    # critical path, so Act stays exp-only.
                    for p_ in range(2):
                        lsl = lrows[s][p_][0:1, qb * 256:(qb + 1) * 256]
                        with nc.allow_low_precision(reason="softmax denom scale"):
                            nc.vector.reciprocal(lsl, lsl)
                        lb = p2.tile([128, 256], BF, name="lb", tag="lb",
                                     bufs=2)
                        nc.gpsimd.partition_broadcast(lb[:], lsl)
                        at = p2.tile([128, 256], BF, name="at", tag="at",
                                     bufs=2)
                        nc.vector.tensor_mul(at[:], ovs[p_][:], lb[:])
                        for e in range(2):
                            hi = AH[p_][s][:, qb, e, :]
                            lo = AL[p_][s][:, qb, e, :]
                            nc.scalar.copy(hi, at[:, e::2])
                            nc.gpsimd.tensor_sub(lo, at[:, e::2], hi)

                def proj_tl(s, tl, last_call=False, dmaq=None):
                    dmaq = dmaq or nc.sync
                    tb = s * 8 + tl
                    for sh in range(2):
                        stg = p2.tile([128, 2048], BF, name="stg",
                                      tag="stg", bufs=2)
                        last = last_call and sh == 1
                        for cc in range(4):
                            chn = sh * 4 + cc
                            oc = psA.tile([128, 512], F32, name="oc",
                                          tag="oc", bufs=4)
                            ti = 0
                            for p_ in range(2):
                                for stat, mov in ((AH, woh_sb), (AH, wol_sb),
                                                  (AL, woh_sb)):
                                    nc.tensor.matmul(
                                        oc[:],
                                        stat[p_][s][:, tl, :, :],
                                        mov[:, p_, :, chn * 512:
                                            (chn + 1) * 512],
                                        start=(ti == 0), stop=(ti == 5),
                                        perf_mode=DR)
                                    ti += 1
                            dsl = stg[:, cc * 512:(cc + 1) * 512]
                            if last and cc == 3:
                                # final chunk: split copy+DMA into halves on
                                # both engines and both queues so the drain
                                # chain (last copy -> last DMA completion)
                                # shortens
                                nc.scalar.copy(dsl[:, 0:256], oc[:, 0:256])
                                nc.gpsimd.dma_start(
                                    outp[tb * 128:(tb + 1) * 128,
                                         chn * 512:chn * 512 + 256],
                                    dsl[:, 0:256])
                                nc.vector.tensor_copy(dsl[:, 256:512],
                                                      oc[:, 256:512])
                                nc.sync.dma_start(
                                    outp[tb * 128:(tb + 1) * 128,
                                         chn * 512 + 256:(chn + 1) * 512],
                                    dsl[:, 256:512])
                                continue
                            if cc % 2 == 0:
                                nc.scalar.copy(dsl, oc[:])
                            else:
                                nc.vector.tensor_copy(dsl, oc[:])
                            if last:
                                # drain fast: per-chunk DMAs right after
                                # each copy, alternating queues; SP (fast
                                # HWDGE) takes the later chunks
                                dq = nc.gpsimd if cc % 2 == 0 else nc.sync
                                dq.dma_start(
                                    outp[tb * 128:(tb + 1) * 128,
                                         chn * 512:(chn + 1) * 512], dsl)
                        if not last:
                            dmaq.dma_start(
                                outp[tb * 128:(tb + 1) * 128,
                                     sh * 2048:(sh + 1) * 2048], stg[:])

                # per block: scores -> the (qb-2) projection (PE filler
                # while exp/mask run; two blocks of slack keeps the
                # multi-engine normalize/split chain off the PE critical
                # path) -> PV -> normalize+split
                # seq 1 ends with qb0: the final block's norm chain (1
                # k-tile) is the shortest, trimming the drain tail
                order = [(0, qb) for qb in range(SEQ // QB)] \
                    + [(1, qb) for qb in range(1, SEQ // QB)] + [(1, 0)]
                # scores run one block ahead so each block's exp/mask chain
                # has a whole iteration to complete before its PV; blocks 0
                # and 1 were pre-emitted inside the ps1 pool
                pts_next = None
                for i, (s, qb) in enumerate(order):
                    pts = pre[i] if i < 2 else pts_next
                    if 1 <= i < len(order) - 1:
                        pts_next = scores_blk(*order[i + 1], psA, "sc", 2)
                    if i >= 2:
                        proj_tl(*order[i - 2])
                    ovs = pv_blk(s, qb, pts)
                    norm_blk(s, qb, ovs)
                # the penultimate projection's big stg DMAs ride the Pool
                # queue so SP is clear for the final fan-out + drain
                proj_tl(*order[-2], dmaq=nc.gpsimd)
                proj_tl(*order[-1], last_call=True)

    nc.compile()
    return nc


def _get_nc():
    if "nc" not in _NC_CACHE:
        _NC_CACHE["nc"] = _build_nc()
    return _NC_CACHE["nc"]


def _pow2_scale(absmax, target=224.0):
    return 2.0 ** np.floor(np.log2(target / max(absmax, 1e-30)))


def _split8(a, scale):
    """-> (hi, lo) e4m3 planes of a*scale (common power-of-2 scale)."""
    s = (a * scale).astype(np.float32)
    hi = s.astype(NPF8)
    lo = (s - hi.astype(np.float32)).astype(NPF8)
    return hi, lo


def _host_prep(x, cos, sin, wq, wk, wv, wo):
    perm = np.concatenate([np.arange(0, 128, 2), np.arange(1, 128, 2)])
    wq_p = wq.reshape(32, 128, DIM)[:, perm, :].reshape(32 * 128, DIM)
    wk_p = wk.reshape(8, 128, DIM)[:, perm, :].reshape(8 * 128, DIM)
    xT = np.ascontiguousarray(x.T)  # [DIM, T]

    sx = _pow2_scale(np.abs(x).max())
    sw = _pow2_scale(max(np.abs(wq).max(), np.abs(wk).max(), np.abs(wv).max()))
    swo = _pow2_scale(np.abs(wo).max())

    xh_, xl_ = _split8(xT, sx)
    # [DIM, T] -> [ch*128+p, kp*1024 + i*512 + c]
    def xlay(a):
        return np.ascontiguousarray(
            a.reshape(NKP, 2, 128, 4, 512).transpose(3, 2, 0, 1, 4)
            .reshape(4 * 128, NKP * 1024))
    xh_, xl_ = xlay(xh_), xlay(xl_)

    # rope tables absorb 1/(sx*sw)
    dsc = 1.0 / (sx * sw)
    cosT = (np.vstack([cos.T, cos.T]) * dsc).astype(NPBF)
    sinT = (np.vstack([sin.T, -sin.T]) * dsc).astype(NPBF)
    p = np.arange(128)[:, None]
    j = np.arange(QB)[None, :]
    masks = [(j >= p).astype(np.float32), (j < p).astype(np.float32)]
    aux = np.concatenate(
        [np.repeat(m, 2, axis=1) for m in masks]
        + [np.zeros((128, 2), np.float32)], axis=1).astype(NPBF)
    vsc = np.full((128, 1), ATT_SCALE * dsc, np.float32)

    in_maps = []
    for c in range(NCORE):
        wqkv = np.concatenate([
            wq_p[c * 512:(c + 1) * 512],
            wk_p[c * 128:(c + 1) * 128],
            wv[c * 128:(c + 1) * 128]], axis=0)  # [768, DIM]
        whh, wll = _split8(wqkv.T, sw)  # [DIM, 768]

        def wlay(a):
            # [p, kp, g, i, m]: group-major within a pair so the first DMA
            # can deliver just pair0/g0
            return np.ascontiguousarray(
                a.reshape(NKP, 2, 128, 6, 128).transpose(2, 0, 3, 1, 4)
                .reshape(128, NKP * 2 * WCOLS))
        wos = wo[:, c * 512:(c + 1) * 512].T  # [512 feat, DIM out]
        woh_, wol_ = _split8(wos, swo)

        def wolay(a):
            return np.ascontiguousarray(
                a.reshape(2, 2, 128, DIM).transpose(2, 0, 1, 3)
                .reshape(128, 2 * 2 * DIM))
        in_maps.append({
            "xh": xh_, "xl": xl_,
            "wh": wlay(whh), "wl": wlay(wll),
            "woh": wolay(woh_), "wol": wolay(wol_),
            "cosT": cosT, "sinT": sinT, "aux": aux, "vsc": vsc,
        })
    return in_maps, 1.0 / (ATT_SCALE * swo)


def kernel(x, cos, sin, wq, wk, wv, wo, n_seqs):
    x = np.asarray(x, dtype=np.float32)
    cos = np.asarray(cos, dtype=np.float32)
    sin = np.asarray(sin, dtype=np.float32)
    wq = np.asarray(wq, dtype=np.float32)
    wk = np.asarray(wk, dtype=np.float32)
    wv = np.asarray(wv, dtype=np.float32)
    wo = np.asarray(wo, dtype=np.float32)
    assert int(n_seqs) == NSEQ and x.shape == (T, DIM)

    nc = _get_nc()
    in_maps, out_dsc = _host_prep(x, cos, sin, wq, wk, wv, wo)
    res = run_bass_kernel_spmd(nc, in_maps, list(range(NCORE))).results
    out = np.zeros((T, DIM), dtype=np.float32)
    for c in range(NCORE):
        out += res[c]["outp"].astype(np.float32)
    return out * out_dsc
